# revision 26
# baseline (speedup 1.0000x reference)
"""Trainium2 Bass kernel for nn_CodeEncoderLayer (sparse-attention transformer
encoder layer).

Sharding: 8 cores = batch (4) x q-token-half (2). Each core independently
computes the full layer for its (batch, 512-query-token) slice. No
collectives; the host shards inputs and concatenates outputs.

Structure notes:
  - All dense projections (q/k/v/pcb, Wo, FFN) run as fp8e4m3 DoubleRow
    matmuls (2 k-tiles per instruction, 0.5 cyc/col) with weights scaled
    x16 on the host and rescaled during PSUM eviction. Scores (QK) and
    PV stay bf16 for softmax precision.
  - The additive attention bias (pm*cb + pmT*pb) is built with 2x-mode
    DVE ops and injected into PSUM together with the mask via identity
    matmuls; QK accumulates on top.
  - Inputs stream over three DMA queues in need-order (each queue
    sustains ~114GB/s); the 2MB of late weights are gated behind the
    v-projection so they don't steal startup bandwidth.

Self-contained: hardcodes E=512, H=8, F=2048, N=1024, B=4.
"""

import numpy as np
import ml_dtypes

E, H, F, N, B = 512, 8, 2048, 1024, 4
HD = E // H          # 64
NQ = 512             # query tokens per core
NCORES = 8
BF = ml_dtypes.bfloat16
F8 = ml_dtypes.float8_e4m3
WS = 16.0            # host-side fp8 weight scale

_CACHE: dict = {}


def _build_nc(zq=True, zk=True, zv=True, zpc=True, zo=True, z2f=True,
              ln1t=True, zb1=True):
    import concourse.bacc as bacc
    import concourse.tile as tile
    from concourse import mybir

    dt = mybir.dt
    AF = mybir.ActivationFunctionType
    OP = mybir.AluOpType
    DR = mybir.MatmulPerfMode.DoubleRow

    nc = bacc.Bacc("TRN2", target_bir_lowering=False, debug=False,
                   num_devices=NCORES)

    def din(name, shape, dtype):
        return nc.dram_tensor(name, list(shape), dtype, kind="ExternalInput")

    # per-core sharded tensors
    xT8_d = din("xT8", (128, 4, N), dt.float8e4)      # x[:,b,:].T chunks (fp8)
    xq_d = din("xq", (128, 4, E), dt.bfloat16)        # x rows for residual
    pm_d = din("pm", (128, 4, N), dt.bfloat16)        # parent_mask[b, qrows, :]
    pmT_d = din("pmT", (128, 4, N), dt.bfloat16)      # parent_mask[b, :, qrows].T
    madd_d = din("madd", (128, 4, N), dt.bfloat16)    # -1e30 * (hidden|pad)
    # shared weights (same array for every core), all x16 in fp8
    wq8_d = din("wq8", (128, 4, E), dt.float8e4)
    wk8_d = din("wk8", (128, 4, E), dt.float8e4)
    wv8_d = din("wv8", (128, 4, E), dt.float8e4)
    wpc8_d = din("wpc8", (128, 4, 16), dt.float8e4)
    wo8_d = din("wo8", (128, 4, E), dt.float8e4)
    w18_d = din("w18", (128, 4, F), dt.float8e4)
    w28h_d = din("w28h", (128, 8, E), dt.float8e4)
    w2b_d = din("w2b", (128, 8, E), dt.bfloat16)
    idb_d = din("idb", (128, 128), dt.bfloat16)       # identity
    # bias tensors (loaded only when nonzero); x16 where they enter psum
    bpc_d = din("bpc", (1, 16), dt.bfloat16)
    bor_d = din("bor", (1, E), dt.bfloat16)
    b2r_d = din("b2r", (1, E), dt.bfloat16)
    bvr_d = din("bvr", (1, E), dt.bfloat16)
    b1c_d = din("b1c", (128, 16, 1), dt.float32)
    b1c16_d = din("b1c16", (128, 16, 1), dt.float32)
    bqc_d = din("bqc", (128, 4, 1), dt.float32)
    bkc_d = din("bkc", (128, 4, 1), dt.float32)
    g1c_d = din("g1c", (128, 4, 1), dt.float32)
    b1lc_d = din("b1lc", (128, 4, 1), dt.float32)
    ones_d = din("ones1", (1, 128), dt.bfloat16)
    sel_d = din("sel", (8, 4, 128), dt.bfloat16)      # head-row selector

    out_d = nc.dram_tensor("out", [4, 128, E], dt.float32, kind="ExternalOutput")

    with tile.TileContext(nc) as tc:
        import contextlib
        stk = contextlib.ExitStack()
        with stk:
            Wp = stk.enter_context(tc.tile_pool(name="persist", bufs=1))
            sm = stk.enter_context(tc.tile_pool(name="small", bufs=4))
            ln = stk.enter_context(tc.tile_pool(name="lnpool", bufs=2))
            sc = stk.enter_context(tc.tile_pool(name="scratch", bufs=3))

            # ---- loads: three DMA queues, ordered by first use ----
            # sync: xT8 half, wq8, wk8, mask qt1 (projection critical path)
            xT8t = Wp.tile([128, 4, N], dt.float8e4, name="xT8t", tag="xT8t")
            nc.sync.dma_start(out=xT8t[:, 0:2, :], in_=xT8_d[:, 0:2, :])
            xT8q = [xT8t[:, 2 * c:2 * c + 2, 0:NQ] for c in range(2)]
            wq8t = Wp.tile([128, 4, E], dt.float8e4, name="wq8t", tag="wq8t")
            nc.sync.dma_start(out=wq8t, in_=wq8_d[:])
            wk8t = Wp.tile([128, 4, E], dt.float8e4, name="wk8t", tag="wk8t")
            nc.sync.dma_start(out=wk8t, in_=wk8_d[:])
            pmTt = Wp.tile([128, 4, N], dt.bfloat16, name="pmTt", tag="pmTt")
            pmt = Wp.tile([128, 4, N], dt.bfloat16, name="pmt", tag="pmt")
            maddt = Wp.tile([128, 4, N], dt.bfloat16, name="maddt",
                            tag="maddt")
            for i in (1,):
                nc.sync.dma_start(out=pmTt[:, i, :], in_=pmT_d[:, i, :])
                nc.sync.dma_start(out=pmt[:, i, :], in_=pm_d[:, i, :])
                nc.sync.dma_start(out=maddt[:, i, :], in_=madd_d[:, i, :])
            # scalar: xT8 other half, wpc8, idb, sel, mask qt0
            nc.scalar.dma_start(out=xT8t[:, 2:4, :], in_=xT8_d[:, 2:4, :])
            wpc8t = Wp.tile([128, 4, 16], dt.float8e4, name="wpc8t",
                            tag="wpc8t")
            nc.scalar.dma_start(out=wpc8t, in_=wpc8_d[:])
            idb = Wp.tile([128, 128], dt.bfloat16, name="idb", tag="idb")
            nc.scalar.dma_start(out=idb, in_=idb_d[:])
            selt = Wp.tile([8, 4, 128], dt.bfloat16, name="selt", tag="selt")
            nc.scalar.dma_start(out=selt, in_=sel_d[:])
            for i in (0,):
                nc.scalar.dma_start(out=pmTt[:, i, :], in_=pmT_d[:, i, :])
                nc.scalar.dma_start(out=pmt[:, i, :], in_=pm_d[:, i, :])
                nc.scalar.dma_start(out=maddt[:, i, :], in_=madd_d[:, i, :])
            # gpsimd: wv8, xq, masks for qt=2/3
            wv8t = Wp.tile([128, 4, E], dt.float8e4, name="wv8t", tag="wv8t")
            nc.gpsimd.dma_start(out=wv8t, in_=wv8_d[:])
            xq_all = Wp.tile([128, 4, E], dt.bfloat16, name="xq_all",
                             tag="xq_all")
            nc.gpsimd.dma_start(out=xq_all, in_=xq_d[:])
            for i in range(2, 4):
                nc.gpsimd.dma_start(out=pmTt[:, i, :], in_=pmT_d[:, i, :])
                nc.gpsimd.dma_start(out=pmt[:, i, :], in_=pm_d[:, i, :])
                nc.gpsimd.dma_start(out=maddt[:, i, :], in_=madd_d[:, i, :])
            pmT = [pmTt[:, i, :] for i in range(4)]
            pm = [pmt[:, i, :] for i in range(4)]
            madd = [maddt[:, i, :] for i in range(4)]
            xq = [xq_all[:, qt, :] for qt in range(4)]

            # small bias tiles (scalar queue), only when actually used
            def sload(dram, shape, dtype, name, n=None):
                if n is None:
                    t = Wp.tile(shape, dtype, name=name, tag=name)
                    nc.scalar.dma_start(out=t, in_=dram[:])
                    return t
                t = Wp.tile([128, n, shape[1]], dtype, name=name, tag=name)
                nc.scalar.dma_start(out=t, in_=dram[:])
                return [t[:, i, :] for i in range(n)]

            need_ones = not (zpc and zv and zo and z2f)
            ones1 = sload(ones_d, [1, 128], dt.bfloat16, "ones1") if need_ones else None
            bpc = sload(bpc_d, [1, 16], dt.bfloat16, "bpc") if not zpc else None
            bqc = sload(bqc_d, [128, 1], dt.float32, "bqc", 4) if not zq else None
            bkc = sload(bkc_d, [128, 1], dt.float32, "bkc", 4) if not zk else None
            bvr = sload(bvr_d, [1, E], dt.bfloat16, "bvr") if not zv else None
            bor = sload(bor_d, [1, E], dt.bfloat16, "bor") if not zo else None
            b2r = sload(b2r_d, [1, E], dt.bfloat16, "b2r") if not z2f else None
            b1c = sload(b1c_d, [128, 1], dt.float32, "b1c", 16) if not zb1 else None
            b1c16 = sload(b1c16_d, [128, 1], dt.float32, "b1c16", 16) if not zb1 else None
            g1c = sload(g1c_d, [128, 1], dt.float32, "g1c", 4) if not ln1t else None
            b1lc = sload(b1lc_d, [128, 1], dt.float32, "b1lc", 4) if not ln1t else None
            eps = Wp.tile([128, 1], dt.float32, name="eps", tag="eps")
            nc.vector.memset(eps, 1e-5)

            MM = nc.tensor.matmul

            def MM8(out, lhsT, rhs, start, stop):
                MM(out, lhsT, rhs, start=start, stop=stop, perf_mode=DR)

            nalt = [0]
            IVS = 1.0 / WS

            def ps2sb(out, ps, scale=None, bias=None):
                """psum->sbuf eviction, alternating DVE/ACT; optional
                (ps*scale)+bias with per-partition bias."""
                nalt[0] += 1
                if scale is not None and bias is not None:
                    nc.vector.tensor_scalar(out, ps, scale, bias,
                                            OP.mult, OP.add)
                elif scale is not None:
                    if nalt[0] % 2 == 0:
                        nc.vector.tensor_scalar(out, ps, scale, None, OP.mult)
                    else:
                        nc.scalar.activation(out, ps, AF.Copy, scale=scale)
                elif nalt[0] % 2 == 0:
                    nc.vector.tensor_copy(out, ps)
                else:
                    nc.scalar.copy(out, ps)

            # ---- Phase A: pcb + q/k/v projections (fp8 DoubleRow) ----
            qT, kT, pcb = [], [], []
            v = [None] * 8
            with tc.tile_pool(name="psA", bufs=2, space="PSUM") as psA:
                for qt in range(4):
                    ps = psA.tile([128, 16], dt.float32, name=f"pspcb{qt}",
                                  tag="pspcb")
                    for c in range(2):
                        MM8(ps, xT8q[c][:, :, qt * 128:(qt + 1) * 128],
                            wpc8t[:, 2 * c:2 * c + 2, :],
                            start=(c == 0), stop=(zpc and c == 1))
                    if not zpc:
                        MM(ps, ones1, bpc, start=False, stop=True)
                    t = Wp.tile([128, 16], dt.float32, name=f"pcb{qt}",
                                tag=f"pcb{qt}")
                    nc.vector.tensor_scalar(t, ps, IVS, None, OP.mult)
                    pcb.append(t)
                for m in range(4):
                    ps = psA.tile([128, NQ], dt.float32, name=f"psq{m}",
                                  tag="psq")
                    for c in range(2):
                        MM8(ps, wq8t[:, 2 * c:2 * c + 2,
                                     m * 128:(m + 1) * 128],
                            xT8q[c], start=(c == 0), stop=(c == 1))
                    t = Wp.tile([128, NQ], dt.bfloat16, name=f"qT{m}",
                                tag=f"qT{m}")
                    # fold 1/sqrt(HD)=1/8 and the 1/16 weight scale here
                    if zq:
                        nc.vector.tensor_scalar(t, ps, IVS / 8.0, None,
                                                OP.mult)
                    else:
                        nc.vector.tensor_scalar(t, ps, IVS / 8.0, bqc[m],
                                                OP.mult, OP.add)
                    qT.append(t)
                    tk = Wp.tile([128, N], dt.bfloat16, name=f"kT{m}",
                                 tag=f"kT{m}")
                    psk = psA.tile([128, N], dt.float32, name=f"psk{m}",
                                   tag="psk")
                    for c in range(2):
                        for tb in range(2):
                            sl = slice(tb * 512, tb * 512 + 512)
                            MM8(psk[:, sl],
                                wk8t[:, 2 * c:2 * c + 2,
                                     m * 128:(m + 1) * 128],
                                xT8t[:, 2 * c:2 * c + 2, sl],
                                start=(c == 0), stop=(c == 1))
                    for tb in range(2):
                        sl = slice(tb * 512, tb * 512 + 512)
                        ps2sb(tk[:, sl], psk[:, sl], scale=IVS,
                              bias=None if zk else bkc[m])
                    kT.append(tk)
                # v-projection
                for tt in range(8):
                    psv = psA.tile([128, E], dt.float32, name=f"psv{tt}",
                                   tag="psq")
                    for c in range(2):
                        MM8(psv, xT8t[:, 2 * c:2 * c + 2,
                                      tt * 128:(tt + 1) * 128],
                            wv8t[:, 2 * c:2 * c + 2, :],
                            start=(c == 0), stop=(zv and c == 1))
                    if not zv:
                        MM(psv, ones1, bvr, start=False, stop=True)
                    t = Wp.tile([128, E], dt.bfloat16, name=f"v{tt}",
                                tag=f"v{tt}")
                    ps2sb(t, psv, scale=IVS)
                    v[tt] = t
            # late weight loads: gated on v so the 2MB doesn't steal
            # DMA bandwidth from the startup-critical loads
            gate = sm.tile([128, 1], dt.bfloat16, name="gate", tag="gate")
            nc.gpsimd.tensor_copy(gate, v[7][:, 0:1])
            wo8t = Wp.tile([128, 4, E], dt.float8e4, name="wo8t", tag="wo8t")
            nc.gpsimd.dma_start(out=wo8t, in_=wo8_d[:])
            w18t = Wp.tile([128, 4, F], dt.float8e4, name="w18t", tag="w18t")
            nc.gpsimd.dma_start(out=w18t, in_=w18_d[:])
            w28ht = Wp.tile([128, 8, E], dt.float8e4, name="w28ht",
                            tag="w28ht")
            nc.gpsimd.dma_start(out=w28ht, in_=w28h_d[:])
            w2bt = Wp.tile([128, 8, E], dt.bfloat16, name="w2bt", tag="w2bt")
            nc.gpsimd.dma_start(out=w2bt, in_=w2b_d[:])
            w2b = [w2bt[:, i, :] for i in range(8)]

            # ---- Phase B: attention (bf16 scores + PV) ----
            ctxT8 = Wp.tile([128, 4, NQ], dt.float8e4, name="ctxT8",
                            tag="ctxT8")
            with (tc.tile_pool(name="psS", bufs=3, space="PSUM") as psS,
                  tc.tile_pool(name="psX", bufs=2, space="PSUM") as psX):
                pT_pend = [None] * 4

                ts_pend = [None] * 4

                def emit_ctx(m):
                    ps_ctx = psX.tile([128, NQ], dt.float32, name=f"psctx{m}",
                                      tag="psctx", bufs=1)
                    pT_all = pT_pend[m]
                    for hh in range(2):
                        h = 2 * m + hh
                        po = hh * 64
                        for kb in range(8):
                            MM(ps_ctx[po:po + 64, :],
                               v[kb][:, h * 64:(h + 1) * 64],
                               pT_all[:, hh * 8 + kb, :], start=(kb == 0),
                               stop=(kb == 7))
                    # normalize whole 2-head block at eviction: ctx * (1/s)
                    nc.vector.tensor_tensor(ctxT8[:, m, :], ps_ctx,
                                            ts_pend[m], OP.mult)

                for m in range(4):
                    # pT_all[:, hh*8+kb, q] = P_raw[h=2m+hh][q, kb*128+p]
                    pT_all = sc.tile([128, 16, NQ], dt.bfloat16,
                                     name=f"pTall{m}", tag="pTall", bufs=2)
                    pT_pend[m] = pT_all
                    smat = sc.tile([128, 128], dt.bfloat16, name=f"smat{m}",
                                   tag="smat", bufs=2)
                    nc.vector.memset(smat, 0.0)
                    for qt in range(4):
                        pn = sc.tile([128, 2, N], dt.bfloat16,
                                     name=f"pn_{m}_{qt}", tag="pn", bufs=2)
                        for hh in range(2):
                            h = 2 * m + hh
                            pb = pcb[qt][:, h:h + 1]
                            cb = pcb[qt][:, 8 + h:9 + h]
                            # am = pm*cb + pmT*pb via fast 2x-mode DVE ops;
                            # madd is injected on the tensor engine
                            t1 = sc.tile([128, N], dt.bfloat16,
                                         name=f"t1_{h}_{qt}", tag=f"t1_{hh}",
                                         bufs=1)
                            nc.vector.tensor_scalar(t1, pm[qt], cb, None,
                                                    OP.mult)
                            t2 = sc.tile([128, N], dt.bfloat16,
                                         name=f"t2_{h}_{qt}", tag=f"t2_{hh}",
                                         bufs=1)
                            nc.vector.tensor_scalar(t2, pmT[qt], pb, None,
                                                    OP.mult)
                            am = sc.tile([128, N], dt.bfloat16,
                                         name=f"am_{h}_{qt}", tag=f"am_{hh}",
                                         bufs=2)
                            nc.vector.tensor_tensor(am, t1, t2, OP.add)
                            ps_s = psS.tile([128, N], dt.float32,
                                            name=f"pss_{h}_{qt}", tag="ps_s")
                            for tb in range(2):
                                sl = slice(tb * 512, tb * 512 + 512)
                                MM(ps_s[:, sl], idb, am[:, sl],
                                   start=True, stop=False)
                                MM(ps_s[:, sl], idb, madd[qt][:, sl],
                                   start=False, stop=False)
                                MM(ps_s[:, sl],
                                   qT[m][hh * 64:hh * 64 + 64,
                                         qt * 128:(qt + 1) * 128],
                                   kT[m][hh * 64:hh * 64 + 64, sl],
                                   start=False, stop=(tb == 1))
                            sums = sm.tile([128, 1], dt.float32,
                                           name=f"sums_{h}_{qt}", tag="sums")
                            nc.scalar.activation(pn[:, hh, :], ps_s, AF.Exp,
                                                 accum_out=sums)
                            with nc.allow_low_precision(
                                    reason="1/s in bf16; uniform per-row "
                                           "scale, tolerance 2e-2"):
                                nc.vector.reciprocal(
                                    smat[:, hh * 4 + qt:hh * 4 + qt + 1],
                                    sums)
                            nc.sync.dma_start_transpose(
                                out=pT_all[:, hh * 8:hh * 8 + 8,
                                           qt * 128:(qt + 1) * 128],
                                in_=pn[:, hh, :])
                    if m > 0:
                        emit_ctx(m - 1)
                    # t_s[p, q] = 1/s_{head(p)}[q], broadcast via matmul
                    smatT = sc.tile([128, 128], dt.bfloat16, name=f"smatT{m}",
                                    tag="smatT", bufs=2)
                    nc.sync.dma_start_transpose(out=smatT, in_=smat)
                    ps_ts = psX.tile([128, NQ], dt.float32, name=f"psts{m}",
                                     tag="psts", bufs=1)
                    for qt in range(4):
                        MM(ps_ts[:, qt * 128:(qt + 1) * 128],
                           selt[:, qt, :], smatT[0:8, :],
                           start=True, stop=True)
                    t_s = sc.tile([128, NQ], dt.float32, name=f"ts{m}",
                                  tag="ts", bufs=2)
                    nc.scalar.copy(t_s, ps_ts)
                    ts_pend[m] = t_s
                emit_ctx(3)

            # ---- Phase C1: Wo + LN1 + y transpose ----
            yb = []
            yT8 = Wp.tile([128, 4, NQ], dt.float8e4, name="yT8", tag="yT8")
            with (tc.tile_pool(name="psAO", bufs=2, space="PSUM") as psAO,
                  tc.tile_pool(name="psYT", bufs=1, space="PSUM") as psYT):
                ps_yT = psYT.tile([128, 4 * NQ], dt.float32, name="ps_yT",
                                  tag="ps_yT")
                for qt in range(4):
                    ps_ao = psAO.tile([128, E], dt.float32, name=f"psao{qt}",
                                      tag="ps_ao")
                    for c in range(2):
                        MM8(ps_ao, ctxT8[:, 2 * c:2 * c + 2,
                                         qt * 128:(qt + 1) * 128],
                            wo8t[:, 2 * c:2 * c + 2, :],
                            start=(c == 0), stop=(zo and c == 1))
                    if not zo:
                        MM(ps_ao, ones1, bor, start=False, stop=True)
                    z = ln.tile([128, E], dt.float32, name=f"z{qt}", tag="z")
                    nc.vector.scalar_tensor_tensor(z, ps_ao, IVS, xq[qt],
                                                   OP.mult, OP.add)
                    stats = sm.tile([128, nc.vector.BN_STATS_DIM], dt.float32,
                                    name=f"stats{qt}", tag="stats")
                    nc.vector.bn_stats(out=stats, in_=z)
                    mv = sm.tile([128, nc.vector.BN_AGGR_DIM], dt.float32,
                                 name=f"mv{qt}", tag="mv")
                    nc.vector.bn_aggr(out=mv, in_=stats)
                    sd = sm.tile([128, 1], dt.float32, name=f"sd{qt}",
                                 tag="sd")
                    nc.scalar.activation(sd, mv[:, 1:2], AF.Sqrt, bias=eps)
                    rstd = sm.tile([128, 1], dt.float32, name=f"rstd{qt}",
                                   tag="rstd")
                    nc.vector.reciprocal(rstd, sd)
                    t = Wp.tile([128, E], dt.bfloat16, name=f"yb{qt}",
                                tag=f"yb{qt}")
                    nc.vector.tensor_scalar(t, z, mv[:, 0:1], rstd,
                                            OP.subtract, OP.mult)
                    yb.append(t)
                    for ec in range(4):
                        MM(ps_yT[:, ec * NQ + qt * 128:
                                 ec * NQ + (qt + 1) * 128],
                           t[:, ec * 128:(ec + 1) * 128], idb,
                           start=True, stop=True)
                for ec in range(4):
                    if ln1t:
                        ps2sb(yT8[:, ec, :], ps_yT[:, ec * NQ:(ec + 1) * NQ])
                    else:
                        nc.vector.tensor_scalar(
                            yT8[:, ec, :], ps_yT[:, ec * NQ:(ec + 1) * NQ],
                            g1c[ec], b1lc[ec], OP.mult, OP.add)

            # ---- Phase C2: FFN + LN2 ----
            h18 = Wp.tile([128, 8, NQ], dt.float8e4, name="h18", tag="h18")
            h1b = []
            with (tc.tile_pool(name="psH", bufs=3, space="PSUM") as psH,
                  tc.tile_pool(name="psF", bufs=2, space="PSUM") as psF):
                for fo in range(16):
                    ps = psH.tile([128, NQ], dt.float32, name=f"psh{fo}",
                                  tag="psH")
                    for c in range(2):
                        MM8(ps, w18t[:, 2 * c:2 * c + 2,
                                     fo * 128:(fo + 1) * 128],
                            yT8[:, 2 * c:2 * c + 2, :],
                            start=(c == 0), stop=(c == 1))
                    if fo < 8:
                        # fp8 h1 at 1x (psum holds 16x)
                        if zb1:
                            if fo % 2 == 0:
                                nc.vector.tensor_scalar(h18[:, fo, :], ps,
                                                        0.0, IVS,
                                                        OP.max, OP.mult)
                            else:
                                nc.scalar.activation(h18[:, fo, :], ps,
                                                     AF.Relu, scale=IVS)
                        else:
                            nc.scalar.activation(h18[:, fo, :], ps, AF.Relu,
                                                 scale=IVS, bias=b1c[fo])
                    else:
                        # bf16 h1 kept at 16x so FFN2 psum scales match
                        th = Wp.tile([128, NQ], dt.bfloat16,
                                     name=f"h1b_{fo}", tag=f"h1b_{fo}")
                        if zb1:
                            if fo % 2 == 0:
                                nc.vector.tensor_scalar(th, ps, 0.0, None,
                                                        OP.max)
                            else:
                                nc.scalar.activation(th, ps, AF.Relu)
                        else:
                            nc.scalar.activation(th, ps, AF.Relu,
                                                 bias=b1c16[fo])
                        h1b.append(th)
                for qt in range(4):
                    ps_ff = psF.tile([128, E], dt.float32, name=f"psff{qt}",
                                     tag="psF")
                    for fc in range(4):
                        MM8(ps_ff, h18[:, 2 * fc:2 * fc + 2,
                                       qt * 128:(qt + 1) * 128],
                            w28ht[:, 2 * fc:2 * fc + 2, :],
                            start=(fc == 0), stop=False)
                    for fc in range(8):
                        MM(ps_ff, h1b[fc][:, qt * 128:(qt + 1) * 128],
                           w2b[fc], start=False, stop=(z2f and fc == 7))
                    if not z2f:
                        MM(ps_ff, ones1, b2r, start=False, stop=True)
                    # residual add folded into the eviction
                    z2 = ln.tile([128, E], dt.float32, name=f"z2_{qt}",
                                 tag="z2")
                    nc.vector.scalar_tensor_tensor(z2, ps_ff, IVS, yb[qt],
                                                   OP.mult, OP.add)
                    stats2 = sm.tile([128, nc.vector.BN_STATS_DIM],
                                     dt.float32, name=f"stats2_{qt}",
                                     tag="stats2")
                    nc.vector.bn_stats(out=stats2, in_=z2)
                    mv2 = sm.tile([128, nc.vector.BN_AGGR_DIM], dt.float32,
                                  name=f"mv2_{qt}", tag="mv2")
                    nc.vector.bn_aggr(out=mv2, in_=stats2)
                    sd2 = sm.tile([128, 1], dt.float32, name=f"sd2_{qt}",
                                  tag="sd2")
                    nc.scalar.activation(sd2, mv2[:, 1:2], AF.Sqrt, bias=eps)
                    rstd2 = sm.tile([128, 1], dt.float32, name=f"rstd2_{qt}",
                                    tag="rstd2")
                    nc.vector.reciprocal(rstd2, sd2)
                    outf = ln.tile([128, E], dt.float32, name=f"outf{qt}",
                                   tag="outf")
                    nc.vector.tensor_scalar(outf, z2, mv2[:, 0:1], rstd2,
                                            OP.subtract, OP.mult)
                    nc.sync.dma_start(out=out_d[qt], in_=outf)

    nc.compile()
    return nc


def _shard(inputs):
    f32 = np.float32
    x = np.asarray(inputs["node_inputs"], f32)
    pmk = np.asarray(inputs["parent_mask"], f32)
    hid = np.asarray(inputs["hidden"]).astype(bool)
    pad = np.asarray(inputs["pad_mask"]).astype(bool)
    Wqkv = np.asarray(inputs["Wqkv"], f32)
    bqkv = np.asarray(inputs["bqkv"], f32)
    Wq, Wk, Wv = Wqkv[:E], Wqkv[E:2 * E], Wqkv[2 * E:]
    bq, bk, bv = bqkv[:E], bqkv[E:2 * E], bqkv[2 * E:]

    def tobf(a):
        return np.ascontiguousarray(a, dtype=f32).astype(BF)

    def to8(a, chunks, width):
        """[E_in, width] -> fp8 x16, chunked [128, chunks, width]."""
        return np.ascontiguousarray(
            (np.ascontiguousarray(a, dtype=f32) * WS).astype(F8)
            .reshape(chunks, 128, width).transpose(1, 0, 2))

    shared = {
        "wq8": to8(Wq.T, 4, E),
        "wk8": to8(Wk.T, 4, E),
        "wv8": to8(Wv.T, 4, E),
        "wpc8": to8(np.concatenate([np.asarray(inputs["Wp"], f32),
                                    np.asarray(inputs["Wc"], f32)], 0).T,
                    4, 16),
        "wo8": to8(np.asarray(inputs["Wo"], f32).T, 4, E),
        "w18": to8(np.asarray(inputs["W1"], f32).T, 4, F),
        "w28h": to8(np.asarray(inputs["W2"], f32)[:, :F // 2].T, 8, E),
        "w2b": np.ascontiguousarray(
            tobf(np.asarray(inputs["W2"], f32)[:, F // 2:].T)
            .reshape(8, 128, E).transpose(1, 0, 2)),
        "bpc": tobf(np.concatenate([np.asarray(inputs["bp"], f32),
                                    np.asarray(inputs["bc"], f32)])[None]
                    * WS),
        "bor": tobf(np.asarray(inputs["bo"], f32)[None] * WS),
        "b2r": tobf(np.asarray(inputs["b2"], f32)[None] * WS),
        "bvr": tobf(bv[None] * WS),
        "b1c": np.ascontiguousarray(
            np.asarray(inputs["b1"], f32).reshape(16, 128, 1).transpose(1, 0, 2)),
        "b1c16": np.ascontiguousarray(
            (np.asarray(inputs["b1"], f32) * WS).reshape(16, 128, 1)
            .transpose(1, 0, 2)),
        "bqc": np.ascontiguousarray((bq / 8.0).reshape(4, 128, 1).transpose(1, 0, 2)),
        "bkc": np.ascontiguousarray(bk.reshape(4, 128, 1).transpose(1, 0, 2)),
        "g1c": np.ascontiguousarray(
            np.asarray(inputs["ln1_g"], f32).reshape(4, 128, 1).transpose(1, 0, 2)),
        "b1lc": np.ascontiguousarray(
            np.asarray(inputs["ln1_b"], f32).reshape(4, 128, 1).transpose(1, 0, 2)),
        "idb": np.eye(128, dtype=BF),
        "ones1": np.ones((1, 128), BF),
        "sel": np.ascontiguousarray(
            (np.arange(8)[:, None, None]
             == (np.arange(128)[None, None, :] // 64) * 4
             + np.arange(4)[None, :, None]).astype(BF)),
    }
    in_maps = []
    for c in range(NCORES):
        b_i, qh = c // 2, c % 2
        qo = qh * NQ
        # key-token permutation: own q-half first (so xT8q == xT8[:, :NQ])
        perm = np.concatenate([np.arange(qo, qo + NQ),
                               np.arange(0, qo), np.arange(qo + NQ, N)])
        xb = x[:, b_i, :]
        xTp = np.ascontiguousarray(xb.T[:, perm], dtype=f32)
        m = dict(shared)
        m["xT8"] = np.ascontiguousarray(
            xTp.astype(F8).reshape(4, 128, N).transpose(1, 0, 2))
        m["xq"] = np.ascontiguousarray(
            xb[qo:qo + NQ].astype(BF).reshape(4, 128, E).transpose(1, 0, 2))
        m["pm"] = np.ascontiguousarray(
            tobf(pmk[b_i][np.ix_(np.arange(qo, qo + NQ), perm)]
                 ).reshape(4, 128, N).transpose(1, 0, 2))
        m["pmT"] = np.ascontiguousarray(
            tobf(pmk[b_i][np.ix_(perm, np.arange(qo, qo + NQ))].T
                 ).reshape(4, 128, N).transpose(1, 0, 2))
        m["madd"] = np.ascontiguousarray(np.where(
            hid[b_i][np.ix_(np.arange(qo, qo + NQ), perm)]
            | pad[b_i][perm][None, :],
            f32(-1e30), f32(0)).astype(BF).reshape(4, 128, N).transpose(1, 0, 2))
        in_maps.append(m)
    return in_maps


def kernel(**inputs):
    from concourse.bass_utils import run_bass_kernel_spmd

    def _z(name):
        return bool(np.all(np.asarray(inputs[name]) == 0))

    flags = dict(
        zq=_z("bqkv"), zk=_z("bqkv"), zv=_z("bqkv"),
        zpc=_z("bp") and _z("bc"), zo=_z("bo"), z2f=_z("b2"),
        zb1=_z("b1"),
        ln1t=bool(np.all(np.asarray(inputs["ln1_g"]) == 1.0)
                  and np.all(np.asarray(inputs["ln1_b"]) == 0.0)))
    key = ("nc",) + tuple(sorted(flags.items()))
    nc = _CACHE.get(key)
    if nc is None:
        nc = _build_nc(**flags)
        _CACHE[key] = nc
    in_maps = _shard(inputs)
    trace = _CACHE.get("trace", False)
    res = run_bass_kernel_spmd(nc, in_maps, core_ids=list(range(NCORES)),
                               trace=trace,
                               tmpdir=_CACHE.get("tmpdir"))
    _CACHE["last_result"] = res

    out = np.zeros((N, B, E), np.float32)
    for c in range(NCORES):
        b_i, qh = c // 2, c % 2
        qo = qh * NQ
        out[qo:qo + NQ, b_i, :] = res.results[c]["out"].reshape(NQ, E)

    g2 = np.asarray(inputs["ln2_g"], np.float32)
    b2l = np.asarray(inputs["ln2_b"], np.float32)
    if not (np.all(g2 == 1.0) and np.all(b2l == 0.0)):
        out = out * g2 + b2l
    return out


# revision 27
# speedup vs baseline: 1.0226x; 1.0226x over previous
"""Trainium2 Bass kernel for nn_CodeEncoderLayer (sparse-attention transformer
encoder layer).

Sharding: 8 cores = batch (4) x q-token-half (2). Each core independently
computes the full layer for its (batch, 512-query-token) slice. No
collectives; the host shards inputs and concatenates outputs.

Structure notes:
  - All dense projections (q/k/v/pcb, Wo, FFN) run as fp8e4m3 DoubleRow
    matmuls (2 k-tiles per instruction, 0.5 cyc/col) with weights scaled
    x16 on the host and rescaled during PSUM eviction. Scores (QK) and
    PV stay bf16 for softmax precision.
  - The additive attention bias (pm*cb + pmT*pb) is built with 2x-mode
    DVE ops and injected into PSUM together with the mask via identity
    matmuls; QK accumulates on top.
  - Inputs stream over three DMA queues in need-order (each queue
    sustains ~114GB/s); the 2MB of late weights are gated behind the
    v-projection so they don't steal startup bandwidth.

Self-contained: hardcodes E=512, H=8, F=2048, N=1024, B=4.
"""

import numpy as np
import ml_dtypes

E, H, F, N, B = 512, 8, 2048, 1024, 4
HD = E // H          # 64
NQ = 512             # query tokens per core
NCORES = 8
BF = ml_dtypes.bfloat16
F8 = ml_dtypes.float8_e4m3
WS = 16.0            # host-side fp8 weight scale

_CACHE: dict = {}


def _build_nc(zq=True, zk=True, zv=True, zpc=True, zo=True, z2f=True,
              ln1t=True, zb1=True):
    import concourse.bacc as bacc
    import concourse.tile as tile
    from concourse import mybir

    dt = mybir.dt
    AF = mybir.ActivationFunctionType
    OP = mybir.AluOpType
    DR = mybir.MatmulPerfMode.DoubleRow

    nc = bacc.Bacc("TRN2", target_bir_lowering=False, debug=False,
                   num_devices=NCORES)

    def din(name, shape, dtype):
        return nc.dram_tensor(name, list(shape), dtype, kind="ExternalInput")

    # per-core sharded tensors
    xT8_d = din("xT8", (128, 4, N), dt.float8e4)      # x[:,b,:].T chunks (fp8)
    xq_d = din("xq", (128, 4, E), dt.bfloat16)        # x rows for residual
    pm_d = din("pm", (128, 4, N), dt.bfloat16)        # parent_mask[b, qrows, :]
    pmT_d = din("pmT", (128, 4, N), dt.bfloat16)      # parent_mask[b, :, qrows].T
    madd_d = din("madd", (128, 4, N), dt.bfloat16)    # -1e30 * (hidden|pad)
    # shared weights (same array for every core), all x16 in fp8
    wq8_d = din("wq8", (128, 4, E), dt.float8e4)
    wk8_d = din("wk8", (128, 4, E), dt.float8e4)
    wv8_d = din("wv8", (128, 4, E), dt.float8e4)
    wpc8_d = din("wpc8", (128, 4, 16), dt.float8e4)
    wo8_d = din("wo8", (128, 4, E), dt.float8e4)
    w18_d = din("w18", (128, 4, F), dt.float8e4)
    w28h_d = din("w28h", (128, 8, E), dt.float8e4)
    w2b_d = din("w2b", (128, 8, E), dt.bfloat16)
    idb_d = din("idb", (128, 128), dt.bfloat16)       # identity
    # bias tensors (loaded only when nonzero); x16 where they enter psum
    bpc_d = din("bpc", (1, 16), dt.bfloat16)
    bor_d = din("bor", (1, E), dt.bfloat16)
    b2r_d = din("b2r", (1, E), dt.bfloat16)
    bvr_d = din("bvr", (1, E), dt.bfloat16)
    b1c_d = din("b1c", (128, 16, 1), dt.float32)
    b1c16_d = din("b1c16", (128, 16, 1), dt.float32)
    bqc_d = din("bqc", (128, 4, 1), dt.float32)
    bkc_d = din("bkc", (128, 4, 1), dt.float32)
    g1c_d = din("g1c", (128, 4, 1), dt.float32)
    b1lc_d = din("b1lc", (128, 4, 1), dt.float32)
    ones_d = din("ones1", (1, 128), dt.bfloat16)
    sel_d = din("sel", (8, 4, 128), dt.bfloat16)      # head-row selector

    out_d = nc.dram_tensor("out", [4, 128, E], dt.float32, kind="ExternalOutput")

    with tile.TileContext(nc) as tc:
        import contextlib
        stk = contextlib.ExitStack()
        with stk:
            Wp = stk.enter_context(tc.tile_pool(name="persist", bufs=1))
            sm = stk.enter_context(tc.tile_pool(name="small", bufs=4))
            ln = stk.enter_context(tc.tile_pool(name="lnpool", bufs=2))
            sc = stk.enter_context(tc.tile_pool(name="scratch", bufs=3))

            # ---- loads: three DMA queues, ordered by first use ----
            # sync: xT8 half, wq8, wk8, mask qt1 (projection critical path)
            xT8t = Wp.tile([128, 4, N], dt.float8e4, name="xT8t", tag="xT8t")
            nc.sync.dma_start(out=xT8t[:, 0:2, :], in_=xT8_d[:, 0:2, :])
            xT8q = [xT8t[:, 2 * c:2 * c + 2, 0:NQ] for c in range(2)]
            wq8t = Wp.tile([128, 4, E], dt.float8e4, name="wq8t", tag="wq8t")
            nc.sync.dma_start(out=wq8t, in_=wq8_d[:])
            wk8t = Wp.tile([128, 4, E], dt.float8e4, name="wk8t", tag="wk8t")
            nc.sync.dma_start(out=wk8t, in_=wk8_d[:])
            pmTt = Wp.tile([128, 4, N], dt.bfloat16, name="pmTt", tag="pmTt")
            pmt = Wp.tile([128, 4, N], dt.bfloat16, name="pmt", tag="pmt")
            maddt = Wp.tile([128, 4, N], dt.bfloat16, name="maddt",
                            tag="maddt")
            for i in (1,):
                nc.sync.dma_start(out=pmTt[:, i, :], in_=pmT_d[:, i, :])
                nc.sync.dma_start(out=pmt[:, i, :], in_=pm_d[:, i, :])
                nc.sync.dma_start(out=maddt[:, i, :], in_=madd_d[:, i, :])
            # scalar: xT8 other half, wpc8, idb, sel, mask qt0
            nc.scalar.dma_start(out=xT8t[:, 2:4, :], in_=xT8_d[:, 2:4, :])
            wpc8t = Wp.tile([128, 4, 16], dt.float8e4, name="wpc8t",
                            tag="wpc8t")
            nc.scalar.dma_start(out=wpc8t, in_=wpc8_d[:])
            idb = Wp.tile([128, 128], dt.bfloat16, name="idb", tag="idb")
            nc.scalar.dma_start(out=idb, in_=idb_d[:])
            selt = Wp.tile([8, 4, 128], dt.bfloat16, name="selt", tag="selt")
            nc.scalar.dma_start(out=selt, in_=sel_d[:])
            for i in (0,):
                nc.scalar.dma_start(out=pmTt[:, i, :], in_=pmT_d[:, i, :])
                nc.scalar.dma_start(out=pmt[:, i, :], in_=pm_d[:, i, :])
                nc.scalar.dma_start(out=maddt[:, i, :], in_=madd_d[:, i, :])
            # gpsimd: wv8, xq, masks for qt=2/3
            wv8t = Wp.tile([128, 4, E], dt.float8e4, name="wv8t", tag="wv8t")
            nc.gpsimd.dma_start(out=wv8t, in_=wv8_d[:])
            xq_all = Wp.tile([128, 4, E], dt.bfloat16, name="xq_all",
                             tag="xq_all")
            nc.gpsimd.dma_start(out=xq_all, in_=xq_d[:])
            for i in range(2, 4):
                nc.gpsimd.dma_start(out=pmTt[:, i, :], in_=pmT_d[:, i, :])
                nc.gpsimd.dma_start(out=pmt[:, i, :], in_=pm_d[:, i, :])
                nc.gpsimd.dma_start(out=maddt[:, i, :], in_=madd_d[:, i, :])
            pmT = [pmTt[:, i, :] for i in range(4)]
            pm = [pmt[:, i, :] for i in range(4)]
            madd = [maddt[:, i, :] for i in range(4)]
            xq = [xq_all[:, qt, :] for qt in range(4)]

            # small bias tiles (scalar queue), only when actually used
            def sload(dram, shape, dtype, name, n=None):
                if n is None:
                    t = Wp.tile(shape, dtype, name=name, tag=name)
                    nc.scalar.dma_start(out=t, in_=dram[:])
                    return t
                t = Wp.tile([128, n, shape[1]], dtype, name=name, tag=name)
                nc.scalar.dma_start(out=t, in_=dram[:])
                return [t[:, i, :] for i in range(n)]

            need_ones = not (zpc and zv and zo and z2f)
            ones1 = sload(ones_d, [1, 128], dt.bfloat16, "ones1") if need_ones else None
            bpc = sload(bpc_d, [1, 16], dt.bfloat16, "bpc") if not zpc else None
            bqc = sload(bqc_d, [128, 1], dt.float32, "bqc", 4) if not zq else None
            bkc = sload(bkc_d, [128, 1], dt.float32, "bkc", 4) if not zk else None
            bvr = sload(bvr_d, [1, E], dt.bfloat16, "bvr") if not zv else None
            bor = sload(bor_d, [1, E], dt.bfloat16, "bor") if not zo else None
            b2r = sload(b2r_d, [1, E], dt.bfloat16, "b2r") if not z2f else None
            b1c = sload(b1c_d, [128, 1], dt.float32, "b1c", 16) if not zb1 else None
            b1c16 = sload(b1c16_d, [128, 1], dt.float32, "b1c16", 16) if not zb1 else None
            g1c = sload(g1c_d, [128, 1], dt.float32, "g1c", 4) if not ln1t else None
            b1lc = sload(b1lc_d, [128, 1], dt.float32, "b1lc", 4) if not ln1t else None
            eps = Wp.tile([128, 1], dt.float32, name="eps", tag="eps")
            nc.vector.memset(eps, 1e-5)

            MM = nc.tensor.matmul

            def MM8(out, lhsT, rhs, start, stop):
                MM(out, lhsT, rhs, start=start, stop=stop, perf_mode=DR)

            nalt = [0]
            IVS = 1.0 / WS

            def ps2sb(out, ps, scale=None, bias=None):
                """psum->sbuf eviction, alternating DVE/ACT; optional
                (ps*scale)+bias with per-partition bias."""
                nalt[0] += 1
                if scale is not None and bias is not None:
                    nc.vector.tensor_scalar(out, ps, scale, bias,
                                            OP.mult, OP.add)
                elif scale is not None:
                    if nalt[0] % 2 == 0:
                        nc.vector.tensor_scalar(out, ps, scale, None, OP.mult)
                    else:
                        nc.scalar.activation(out, ps, AF.Copy, scale=scale)
                elif nalt[0] % 2 == 0:
                    nc.vector.tensor_copy(out, ps)
                else:
                    nc.scalar.copy(out, ps)

            # ---- Phase A: pcb + q/k/v projections (fp8 DoubleRow) ----
            qT, kT, pcb = [], [], []
            v = [None] * 8
            with tc.tile_pool(name="psA", bufs=2, space="PSUM") as psA:
                for qt in range(4):
                    ps = psA.tile([128, 16], dt.float32, name=f"pspcb{qt}",
                                  tag="pspcb")
                    for c in range(2):
                        MM8(ps, xT8q[c][:, :, qt * 128:(qt + 1) * 128],
                            wpc8t[:, 2 * c:2 * c + 2, :],
                            start=(c == 0), stop=(zpc and c == 1))
                    if not zpc:
                        MM(ps, ones1, bpc, start=False, stop=True)
                    t = Wp.tile([128, 16], dt.float32, name=f"pcb{qt}",
                                tag=f"pcb{qt}")
                    nc.vector.tensor_scalar(t, ps, IVS, None, OP.mult)
                    pcb.append(t)
                for m in range(4):
                    ps = psA.tile([128, NQ], dt.float32, name=f"psq{m}",
                                  tag="psq")
                    for c in range(2):
                        MM8(ps, wq8t[:, 2 * c:2 * c + 2,
                                     m * 128:(m + 1) * 128],
                            xT8q[c], start=(c == 0), stop=(c == 1))
                    t = Wp.tile([128, NQ], dt.bfloat16, name=f"qT{m}",
                                tag=f"qT{m}")
                    # fold 1/sqrt(HD)=1/8 and the 1/16 weight scale here
                    if zq:
                        nc.vector.tensor_scalar(t, ps, IVS / 8.0, None,
                                                OP.mult)
                    else:
                        nc.vector.tensor_scalar(t, ps, IVS / 8.0, bqc[m],
                                                OP.mult, OP.add)
                    qT.append(t)
                    tk = Wp.tile([128, N], dt.bfloat16, name=f"kT{m}",
                                 tag=f"kT{m}")
                    psk = psA.tile([128, N], dt.float32, name=f"psk{m}",
                                   tag="psk")
                    for c in range(2):
                        for tb in range(2):
                            sl = slice(tb * 512, tb * 512 + 512)
                            MM8(psk[:, sl],
                                wk8t[:, 2 * c:2 * c + 2,
                                     m * 128:(m + 1) * 128],
                                xT8t[:, 2 * c:2 * c + 2, sl],
                                start=(c == 0), stop=(c == 1))
                    for tb in range(2):
                        sl = slice(tb * 512, tb * 512 + 512)
                        ps2sb(tk[:, sl], psk[:, sl], scale=IVS,
                              bias=None if zk else bkc[m])
                    kT.append(tk)
                # v-projection
                for tt in range(8):
                    psv = psA.tile([128, E], dt.float32, name=f"psv{tt}",
                                   tag="psq")
                    for c in range(2):
                        MM8(psv, xT8t[:, 2 * c:2 * c + 2,
                                      tt * 128:(tt + 1) * 128],
                            wv8t[:, 2 * c:2 * c + 2, :],
                            start=(c == 0), stop=(zv and c == 1))
                    if not zv:
                        MM(psv, ones1, bvr, start=False, stop=True)
                    t = Wp.tile([128, E], dt.bfloat16, name=f"v{tt}",
                                tag=f"v{tt}")
                    ps2sb(t, psv, scale=IVS)
                    v[tt] = t
            # late weight loads: gated on v so the 2MB doesn't steal
            # DMA bandwidth from the startup-critical loads
            gate = sm.tile([128, 1], dt.bfloat16, name="gate", tag="gate")
            nc.gpsimd.tensor_copy(gate, v[7][:, 0:1])
            wo8t = Wp.tile([128, 4, E], dt.float8e4, name="wo8t", tag="wo8t")
            nc.gpsimd.dma_start(out=wo8t, in_=wo8_d[:])
            w18t = Wp.tile([128, 4, F], dt.float8e4, name="w18t", tag="w18t")
            nc.gpsimd.dma_start(out=w18t, in_=w18_d[:])
            w28ht = Wp.tile([128, 8, E], dt.float8e4, name="w28ht",
                            tag="w28ht")
            nc.gpsimd.dma_start(out=w28ht, in_=w28h_d[:])
            w2bt = Wp.tile([128, 8, E], dt.bfloat16, name="w2bt", tag="w2bt")
            nc.gpsimd.dma_start(out=w2bt, in_=w2b_d[:])
            w2b = [w2bt[:, i, :] for i in range(8)]

            # ---- Phase B: attention (bf16 scores + PV) ----
            ctxT8 = Wp.tile([128, 4, NQ], dt.float8e4, name="ctxT8",
                            tag="ctxT8")
            with (tc.tile_pool(name="psS", bufs=3, space="PSUM") as psS,
                  tc.tile_pool(name="psX", bufs=2, space="PSUM") as psX):
                pT_pend = [None] * 4

                ts_pend = [None] * 4

                def emit_ctx(m):
                    ps_ctx = psX.tile([128, NQ], dt.float32, name=f"psctx{m}",
                                      tag="psctx", bufs=1)
                    pT_all = pT_pend[m]
                    for hh in range(2):
                        h = 2 * m + hh
                        po = hh * 64
                        for kb in range(8):
                            MM(ps_ctx[po:po + 64, :],
                               v[kb][:, h * 64:(h + 1) * 64],
                               pT_all[:, hh * 8 + kb, :], start=(kb == 0),
                               stop=(kb == 7))
                    # normalize whole 2-head block at eviction: ctx * (1/s)
                    nc.vector.tensor_tensor(ctxT8[:, m, :], ps_ctx,
                                            ts_pend[m], OP.mult)

                for m in range(4):
                    # pT_all[:, hh*8+kb, q] = P_raw[h=2m+hh][q, kb*128+p]
                    pT_all = sc.tile([128, 16, NQ], dt.bfloat16,
                                     name=f"pTall{m}", tag="pTall", bufs=2)
                    pT_pend[m] = pT_all
                    smat = sc.tile([128, 128], dt.bfloat16, name=f"smat{m}",
                                   tag="smat", bufs=2)
                    nc.vector.memset(smat, 0.0)
                    for qt in range(4):
                        pn = sc.tile([128, 2, N], dt.bfloat16,
                                     name=f"pn_{m}_{qt}", tag="pn", bufs=2)
                        for hh in range(2):
                            h = 2 * m + hh
                            pb = pcb[qt][:, h:h + 1]
                            cb = pcb[qt][:, 8 + h:9 + h]
                            # am = pm*cb + pmT*pb via fast 2x-mode DVE ops;
                            # madd is injected on the tensor engine
                            t1 = sc.tile([128, N], dt.bfloat16,
                                         name=f"t1_{h}_{qt}", tag=f"t1_{hh}",
                                         bufs=1)
                            nc.vector.tensor_scalar(t1, pm[qt], cb, None,
                                                    OP.mult)
                            ps_s = psS.tile([128, N], dt.float32,
                                            name=f"pss_{h}_{qt}", tag="ps_s")
                            if hh == 0 and qt in (1, 2):
                                # preload bias+mask into PSUM on the DVE to
                                # relieve the tensor engine (it is the
                                # bottleneck); QK accumulates on top
                                t2m = sc.tile([128, N], dt.bfloat16,
                                              name=f"t2m_{h}_{qt}",
                                              tag="t2m", bufs=2)
                                nc.vector.scalar_tensor_tensor(
                                    t2m, pmT[qt], pb, madd[qt],
                                    OP.mult, OP.add)
                                nc.vector.tensor_tensor(ps_s, t1, t2m,
                                                        OP.add)
                                for tb in range(2):
                                    sl = slice(tb * 512, tb * 512 + 512)
                                    MM(ps_s[:, sl],
                                       qT[m][hh * 64:hh * 64 + 64,
                                             qt * 128:(qt + 1) * 128],
                                       kT[m][hh * 64:hh * 64 + 64, sl],
                                       start=False, stop=(tb == 1),
                                       skip_group_check=True)
                            else:
                                t2 = sc.tile([128, N], dt.bfloat16,
                                             name=f"t2_{h}_{qt}",
                                             tag=f"t2_{hh}", bufs=1)
                                nc.vector.tensor_scalar(t2, pmT[qt], pb,
                                                        None, OP.mult)
                                am = sc.tile([128, N], dt.bfloat16,
                                             name=f"am_{h}_{qt}",
                                             tag=f"am_{hh}", bufs=2)
                                nc.vector.tensor_tensor(am, t1, t2, OP.add)
                                for tb in range(2):
                                    sl = slice(tb * 512, tb * 512 + 512)
                                    MM(ps_s[:, sl], idb, am[:, sl],
                                       start=True, stop=False)
                                    MM(ps_s[:, sl], idb, madd[qt][:, sl],
                                       start=False, stop=False)
                                    MM(ps_s[:, sl],
                                       qT[m][hh * 64:hh * 64 + 64,
                                             qt * 128:(qt + 1) * 128],
                                       kT[m][hh * 64:hh * 64 + 64, sl],
                                       start=False, stop=(tb == 1))
                            sums = sm.tile([128, 1], dt.float32,
                                           name=f"sums_{h}_{qt}", tag="sums")
                            nc.scalar.activation(pn[:, hh, :], ps_s, AF.Exp,
                                                 accum_out=sums)
                            with nc.allow_low_precision(
                                    reason="1/s in bf16; uniform per-row "
                                           "scale, tolerance 2e-2"):
                                nc.vector.reciprocal(
                                    smat[:, hh * 4 + qt:hh * 4 + qt + 1],
                                    sums)
                            nc.sync.dma_start_transpose(
                                out=pT_all[:, hh * 8:hh * 8 + 8,
                                           qt * 128:(qt + 1) * 128],
                                in_=pn[:, hh, :])
                    if m > 0:
                        emit_ctx(m - 1)
                    # t_s[p, q] = 1/s_{head(p)}[q], broadcast via matmul
                    smatT = sc.tile([128, 128], dt.bfloat16, name=f"smatT{m}",
                                    tag="smatT", bufs=2)
                    nc.sync.dma_start_transpose(out=smatT, in_=smat)
                    ps_ts = psX.tile([128, NQ], dt.float32, name=f"psts{m}",
                                     tag="psts", bufs=1)
                    for qt in range(4):
                        MM(ps_ts[:, qt * 128:(qt + 1) * 128],
                           selt[:, qt, :], smatT[0:8, :],
                           start=True, stop=True)
                    t_s = sc.tile([128, NQ], dt.float32, name=f"ts{m}",
                                  tag="ts", bufs=2)
                    nc.scalar.copy(t_s, ps_ts)
                    ts_pend[m] = t_s
                emit_ctx(3)

            # ---- Phase C1: Wo + LN1 + y transpose ----
            yb = []
            yT8 = Wp.tile([128, 4, NQ], dt.float8e4, name="yT8", tag="yT8")
            with (tc.tile_pool(name="psAO", bufs=2, space="PSUM") as psAO,
                  tc.tile_pool(name="psYT", bufs=1, space="PSUM") as psYT):
                ps_yT = psYT.tile([128, 4 * NQ], dt.float32, name="ps_yT",
                                  tag="ps_yT")
                for qt in range(4):
                    ps_ao = psAO.tile([128, E], dt.float32, name=f"psao{qt}",
                                      tag="ps_ao")
                    for c in range(2):
                        MM8(ps_ao, ctxT8[:, 2 * c:2 * c + 2,
                                         qt * 128:(qt + 1) * 128],
                            wo8t[:, 2 * c:2 * c + 2, :],
                            start=(c == 0), stop=(zo and c == 1))
                    if not zo:
                        MM(ps_ao, ones1, bor, start=False, stop=True)
                    z = ln.tile([128, E], dt.float32, name=f"z{qt}", tag="z")
                    nc.vector.scalar_tensor_tensor(z, ps_ao, IVS, xq[qt],
                                                   OP.mult, OP.add)
                    stats = sm.tile([128, nc.vector.BN_STATS_DIM], dt.float32,
                                    name=f"stats{qt}", tag="stats")
                    nc.vector.bn_stats(out=stats, in_=z)
                    mv = sm.tile([128, nc.vector.BN_AGGR_DIM], dt.float32,
                                 name=f"mv{qt}", tag="mv")
                    nc.vector.bn_aggr(out=mv, in_=stats)
                    sd = sm.tile([128, 1], dt.float32, name=f"sd{qt}",
                                 tag="sd")
                    nc.scalar.activation(sd, mv[:, 1:2], AF.Sqrt, bias=eps)
                    rstd = sm.tile([128, 1], dt.float32, name=f"rstd{qt}",
                                   tag="rstd")
                    nc.vector.reciprocal(rstd, sd)
                    t = Wp.tile([128, E], dt.bfloat16, name=f"yb{qt}",
                                tag=f"yb{qt}")
                    nc.vector.tensor_scalar(t, z, mv[:, 0:1], rstd,
                                            OP.subtract, OP.mult)
                    yb.append(t)
                    for ec in range(4):
                        MM(ps_yT[:, ec * NQ + qt * 128:
                                 ec * NQ + (qt + 1) * 128],
                           t[:, ec * 128:(ec + 1) * 128], idb,
                           start=True, stop=True)
                for ec in range(4):
                    if ln1t:
                        ps2sb(yT8[:, ec, :], ps_yT[:, ec * NQ:(ec + 1) * NQ])
                    else:
                        nc.vector.tensor_scalar(
                            yT8[:, ec, :], ps_yT[:, ec * NQ:(ec + 1) * NQ],
                            g1c[ec], b1lc[ec], OP.mult, OP.add)

            # ---- Phase C2: FFN + LN2 ----
            h18 = Wp.tile([128, 8, NQ], dt.float8e4, name="h18", tag="h18")
            h1b = []
            with (tc.tile_pool(name="psH", bufs=3, space="PSUM") as psH,
                  tc.tile_pool(name="psF", bufs=2, space="PSUM") as psF):
                for fo in range(16):
                    ps = psH.tile([128, NQ], dt.float32, name=f"psh{fo}",
                                  tag="psH")
                    for c in range(2):
                        MM8(ps, w18t[:, 2 * c:2 * c + 2,
                                     fo * 128:(fo + 1) * 128],
                            yT8[:, 2 * c:2 * c + 2, :],
                            start=(c == 0), stop=(c == 1))
                    if fo < 8:
                        # fp8 h1 at 1x (psum holds 16x)
                        if zb1:
                            if fo % 2 == 0:
                                nc.vector.tensor_scalar(h18[:, fo, :], ps,
                                                        0.0, IVS,
                                                        OP.max, OP.mult)
                            else:
                                nc.scalar.activation(h18[:, fo, :], ps,
                                                     AF.Relu, scale=IVS)
                        else:
                            nc.scalar.activation(h18[:, fo, :], ps, AF.Relu,
                                                 scale=IVS, bias=b1c[fo])
                    else:
                        # bf16 h1 kept at 16x so FFN2 psum scales match
                        th = Wp.tile([128, NQ], dt.bfloat16,
                                     name=f"h1b_{fo}", tag=f"h1b_{fo}")
                        if zb1:
                            if fo % 2 == 0:
                                nc.vector.tensor_scalar(th, ps, 0.0, None,
                                                        OP.max)
                            else:
                                nc.scalar.activation(th, ps, AF.Relu)
                        else:
                            nc.scalar.activation(th, ps, AF.Relu,
                                                 bias=b1c16[fo])
                        h1b.append(th)
                for qt in range(4):
                    ps_ff = psF.tile([128, E], dt.float32, name=f"psff{qt}",
                                     tag="psF")
                    for fc in range(4):
                        MM8(ps_ff, h18[:, 2 * fc:2 * fc + 2,
                                       qt * 128:(qt + 1) * 128],
                            w28ht[:, 2 * fc:2 * fc + 2, :],
                            start=(fc == 0), stop=False)
                    for fc in range(8):
                        MM(ps_ff, h1b[fc][:, qt * 128:(qt + 1) * 128],
                           w2b[fc], start=False, stop=(z2f and fc == 7))
                    if not z2f:
                        MM(ps_ff, ones1, b2r, start=False, stop=True)
                    # residual add folded into the eviction
                    z2 = ln.tile([128, E], dt.float32, name=f"z2_{qt}",
                                 tag="z2")
                    nc.vector.scalar_tensor_tensor(z2, ps_ff, IVS, yb[qt],
                                                   OP.mult, OP.add)
                    stats2 = sm.tile([128, nc.vector.BN_STATS_DIM],
                                     dt.float32, name=f"stats2_{qt}",
                                     tag="stats2")
                    nc.vector.bn_stats(out=stats2, in_=z2)
                    mv2 = sm.tile([128, nc.vector.BN_AGGR_DIM], dt.float32,
                                  name=f"mv2_{qt}", tag="mv2")
                    nc.vector.bn_aggr(out=mv2, in_=stats2)
                    sd2 = sm.tile([128, 1], dt.float32, name=f"sd2_{qt}",
                                  tag="sd2")
                    nc.scalar.activation(sd2, mv2[:, 1:2], AF.Sqrt, bias=eps)
                    rstd2 = sm.tile([128, 1], dt.float32, name=f"rstd2_{qt}",
                                    tag="rstd2")
                    nc.vector.reciprocal(rstd2, sd2)
                    outf = ln.tile([128, E], dt.float32, name=f"outf{qt}",
                                   tag="outf")
                    nc.vector.tensor_scalar(outf, z2, mv2[:, 0:1], rstd2,
                                            OP.subtract, OP.mult)
                    nc.sync.dma_start(out=out_d[qt], in_=outf)

    nc.compile()
    return nc


def _shard(inputs):
    f32 = np.float32
    x = np.asarray(inputs["node_inputs"], f32)
    pmk = np.asarray(inputs["parent_mask"], f32)
    hid = np.asarray(inputs["hidden"]).astype(bool)
    pad = np.asarray(inputs["pad_mask"]).astype(bool)
    Wqkv = np.asarray(inputs["Wqkv"], f32)
    bqkv = np.asarray(inputs["bqkv"], f32)
    Wq, Wk, Wv = Wqkv[:E], Wqkv[E:2 * E], Wqkv[2 * E:]
    bq, bk, bv = bqkv[:E], bqkv[E:2 * E], bqkv[2 * E:]

    def tobf(a):
        return np.ascontiguousarray(a, dtype=f32).astype(BF)

    def to8(a, chunks, width):
        """[E_in, width] -> fp8 x16, chunked [128, chunks, width]."""
        return np.ascontiguousarray(
            (np.ascontiguousarray(a, dtype=f32) * WS).astype(F8)
            .reshape(chunks, 128, width).transpose(1, 0, 2))

    shared = {
        "wq8": to8(Wq.T, 4, E),
        "wk8": to8(Wk.T, 4, E),
        "wv8": to8(Wv.T, 4, E),
        "wpc8": to8(np.concatenate([np.asarray(inputs["Wp"], f32),
                                    np.asarray(inputs["Wc"], f32)], 0).T,
                    4, 16),
        "wo8": to8(np.asarray(inputs["Wo"], f32).T, 4, E),
        "w18": to8(np.asarray(inputs["W1"], f32).T, 4, F),
        "w28h": to8(np.asarray(inputs["W2"], f32)[:, :F // 2].T, 8, E),
        "w2b": np.ascontiguousarray(
            tobf(np.asarray(inputs["W2"], f32)[:, F // 2:].T)
            .reshape(8, 128, E).transpose(1, 0, 2)),
        "bpc": tobf(np.concatenate([np.asarray(inputs["bp"], f32),
                                    np.asarray(inputs["bc"], f32)])[None]
                    * WS),
        "bor": tobf(np.asarray(inputs["bo"], f32)[None] * WS),
        "b2r": tobf(np.asarray(inputs["b2"], f32)[None] * WS),
        "bvr": tobf(bv[None] * WS),
        "b1c": np.ascontiguousarray(
            np.asarray(inputs["b1"], f32).reshape(16, 128, 1).transpose(1, 0, 2)),
        "b1c16": np.ascontiguousarray(
            (np.asarray(inputs["b1"], f32) * WS).reshape(16, 128, 1)
            .transpose(1, 0, 2)),
        "bqc": np.ascontiguousarray((bq / 8.0).reshape(4, 128, 1).transpose(1, 0, 2)),
        "bkc": np.ascontiguousarray(bk.reshape(4, 128, 1).transpose(1, 0, 2)),
        "g1c": np.ascontiguousarray(
            np.asarray(inputs["ln1_g"], f32).reshape(4, 128, 1).transpose(1, 0, 2)),
        "b1lc": np.ascontiguousarray(
            np.asarray(inputs["ln1_b"], f32).reshape(4, 128, 1).transpose(1, 0, 2)),
        "idb": np.eye(128, dtype=BF),
        "ones1": np.ones((1, 128), BF),
        "sel": np.ascontiguousarray(
            (np.arange(8)[:, None, None]
             == (np.arange(128)[None, None, :] // 64) * 4
             + np.arange(4)[None, :, None]).astype(BF)),
    }
    in_maps = []
    for c in range(NCORES):
        b_i, qh = c // 2, c % 2
        qo = qh * NQ
        # key-token permutation: own q-half first (so xT8q == xT8[:, :NQ])
        perm = np.concatenate([np.arange(qo, qo + NQ),
                               np.arange(0, qo), np.arange(qo + NQ, N)])
        xb = x[:, b_i, :]
        xTp = np.ascontiguousarray(xb.T[:, perm], dtype=f32)
        m = dict(shared)
        m["xT8"] = np.ascontiguousarray(
            xTp.astype(F8).reshape(4, 128, N).transpose(1, 0, 2))
        m["xq"] = np.ascontiguousarray(
            xb[qo:qo + NQ].astype(BF).reshape(4, 128, E).transpose(1, 0, 2))
        m["pm"] = np.ascontiguousarray(
            tobf(pmk[b_i][np.ix_(np.arange(qo, qo + NQ), perm)]
                 ).reshape(4, 128, N).transpose(1, 0, 2))
        m["pmT"] = np.ascontiguousarray(
            tobf(pmk[b_i][np.ix_(perm, np.arange(qo, qo + NQ))].T
                 ).reshape(4, 128, N).transpose(1, 0, 2))
        m["madd"] = np.ascontiguousarray(np.where(
            hid[b_i][np.ix_(np.arange(qo, qo + NQ), perm)]
            | pad[b_i][perm][None, :],
            f32(-1e30), f32(0)).astype(BF).reshape(4, 128, N).transpose(1, 0, 2))
        in_maps.append(m)
    return in_maps


def kernel(**inputs):
    from concourse.bass_utils import run_bass_kernel_spmd

    def _z(name):
        return bool(np.all(np.asarray(inputs[name]) == 0))

    flags = dict(
        zq=_z("bqkv"), zk=_z("bqkv"), zv=_z("bqkv"),
        zpc=_z("bp") and _z("bc"), zo=_z("bo"), z2f=_z("b2"),
        zb1=_z("b1"),
        ln1t=bool(np.all(np.asarray(inputs["ln1_g"]) == 1.0)
                  and np.all(np.asarray(inputs["ln1_b"]) == 0.0)))
    key = ("nc",) + tuple(sorted(flags.items()))
    nc = _CACHE.get(key)
    if nc is None:
        nc = _build_nc(**flags)
        _CACHE[key] = nc
    in_maps = _shard(inputs)
    trace = _CACHE.get("trace", False)
    res = run_bass_kernel_spmd(nc, in_maps, core_ids=list(range(NCORES)),
                               trace=trace,
                               tmpdir=_CACHE.get("tmpdir"))
    _CACHE["last_result"] = res

    out = np.zeros((N, B, E), np.float32)
    for c in range(NCORES):
        b_i, qh = c // 2, c % 2
        qo = qh * NQ
        out[qo:qo + NQ, b_i, :] = res.results[c]["out"].reshape(NQ, E)

    g2 = np.asarray(inputs["ln2_g"], np.float32)
    b2l = np.asarray(inputs["ln2_b"], np.float32)
    if not (np.all(g2 == 1.0) and np.all(b2l == 0.0)):
        out = out * g2 + b2l
    return out


# revision 28
# speedup vs baseline: 1.0652x; 1.0416x over previous
"""Trainium2 Bass kernel for nn_CodeEncoderLayer (sparse-attention transformer
encoder layer).

Sharding: 8 cores = batch (4) x q-token-half (2). Each core independently
computes the full layer for its (batch, 512-query-token) slice. No
collectives; the host shards inputs and concatenates outputs.

Structure notes:
  - All dense projections (q/k/v/pcb, Wo, FFN) run as fp8e4m3 DoubleRow
    matmuls (2 k-tiles per instruction, 0.5 cyc/col) with weights scaled
    x16 on the host and rescaled during PSUM eviction. Scores (QK) and
    PV stay bf16 for softmax precision.
  - The additive attention bias (pm*cb + pmT*pb) is built with 2x-mode
    DVE ops and injected into PSUM together with the mask via identity
    matmuls; QK accumulates on top.
  - Inputs stream over three DMA queues in need-order (each queue
    sustains ~114GB/s); the 2MB of late weights are gated behind the
    v-projection so they don't steal startup bandwidth.

Self-contained: hardcodes E=512, H=8, F=2048, N=1024, B=4.
"""

import numpy as np
import ml_dtypes

E, H, F, N, B = 512, 8, 2048, 1024, 4
HD = E // H          # 64
NQ = 512             # query tokens per core
NCORES = 8
BF = ml_dtypes.bfloat16
F8 = ml_dtypes.float8_e4m3
WS = 16.0            # host-side fp8 weight scale

_CACHE: dict = {}


def _build_nc(zq=True, zk=True, zv=True, zpc=True, zo=True, z2f=True,
              ln1t=True, zb1=True):
    import concourse.bacc as bacc
    import concourse.tile as tile
    from concourse import mybir

    dt = mybir.dt
    AF = mybir.ActivationFunctionType
    OP = mybir.AluOpType
    DR = mybir.MatmulPerfMode.DoubleRow

    nc = bacc.Bacc("TRN2", target_bir_lowering=False, debug=False,
                   num_devices=NCORES)

    def din(name, shape, dtype):
        return nc.dram_tensor(name, list(shape), dtype, kind="ExternalInput")

    # per-core sharded tensors
    xT8_d = din("xT8", (128, 4, N), dt.float8e4)      # x[:,b,:].T chunks (fp8)
    xq_d = din("xq", (128, 4, E), dt.bfloat16)        # x rows for residual
    pm_d = din("pm", (128, 4, N), dt.bfloat16)        # parent_mask[b, qrows, :]
    pmT_d = din("pmT", (128, 4, N), dt.bfloat16)      # parent_mask[b, :, qrows].T
    madd_d = din("madd", (128, 4, N), dt.bfloat16)    # -1e30 * (hidden|pad)
    # shared weights (same array for every core), all x16 in fp8
    wq8_d = din("wq8", (128, 4, E), dt.float8e4)
    wk8_d = din("wk8", (128, 4, E), dt.float8e4)
    wv8_d = din("wv8", (128, 4, E), dt.float8e4)
    wpc8_d = din("wpc8", (128, 4, 16), dt.float8e4)
    wo8_d = din("wo8", (128, 4, E), dt.float8e4)
    w18_d = din("w18", (128, 4, F), dt.float8e4)
    w28h_d = din("w28h", (128, 8, E), dt.float8e4)
    w2b_d = din("w2b", (128, 8, E), dt.bfloat16)
    idb_d = din("idb", (128, 128), dt.bfloat16)       # identity
    # bias tensors (loaded only when nonzero); x16 where they enter psum
    bpc_d = din("bpc", (1, 16), dt.bfloat16)
    bor_d = din("bor", (1, E), dt.bfloat16)
    b2r_d = din("b2r", (1, E), dt.bfloat16)
    bvr_d = din("bvr", (1, E), dt.bfloat16)
    b1c_d = din("b1c", (128, 16, 1), dt.float32)
    b1c16_d = din("b1c16", (128, 16, 1), dt.float32)
    bqc_d = din("bqc", (128, 4, 1), dt.float32)
    bkc_d = din("bkc", (128, 4, 1), dt.float32)
    g1c_d = din("g1c", (128, 4, 1), dt.float32)
    b1lc_d = din("b1lc", (128, 4, 1), dt.float32)
    ones_d = din("ones1", (1, 128), dt.bfloat16)
    sel_d = din("sel", (8, 4, 128), dt.bfloat16)      # head-row selector

    out_d = nc.dram_tensor("out", [4, 128, E], dt.float32, kind="ExternalOutput")

    with tile.TileContext(nc) as tc:
        import contextlib
        stk = contextlib.ExitStack()
        with stk:
            Wp = stk.enter_context(tc.tile_pool(name="persist", bufs=1))
            sm = stk.enter_context(tc.tile_pool(name="small", bufs=4))
            ln = stk.enter_context(tc.tile_pool(name="lnpool", bufs=2))
            sc = stk.enter_context(tc.tile_pool(name="scratch", bufs=3))

            # ---- loads: three DMA queues, ordered by first use ----
            # sync: xT8 half, wq8, wk8, mask qt1 (projection critical path)
            xT8t = Wp.tile([128, 4, N], dt.float8e4, name="xT8t", tag="xT8t")
            nc.sync.dma_start(out=xT8t[:, 0:2, :], in_=xT8_d[:, 0:2, :])
            xT8q = [xT8t[:, 2 * c:2 * c + 2, 0:NQ] for c in range(2)]
            wq8t = Wp.tile([128, 4, E], dt.float8e4, name="wq8t", tag="wq8t")
            nc.sync.dma_start(out=wq8t, in_=wq8_d[:])
            wk8t = Wp.tile([128, 4, E], dt.float8e4, name="wk8t", tag="wk8t")
            nc.sync.dma_start(out=wk8t, in_=wk8_d[:])
            pmTt = Wp.tile([128, 4, N], dt.bfloat16, name="pmTt", tag="pmTt")
            pmt = Wp.tile([128, 4, N], dt.bfloat16, name="pmt", tag="pmt")
            maddt = Wp.tile([128, 4, N], dt.bfloat16, name="maddt",
                            tag="maddt")
            for i in (1,):
                nc.sync.dma_start(out=pmTt[:, i, :], in_=pmT_d[:, i, :])
                nc.sync.dma_start(out=pmt[:, i, :], in_=pm_d[:, i, :])
                nc.sync.dma_start(out=maddt[:, i, :], in_=madd_d[:, i, :])
            # scalar: xT8 other half, wpc8, idb, sel, mask qt0
            nc.scalar.dma_start(out=xT8t[:, 2:4, :], in_=xT8_d[:, 2:4, :])
            wpc8t = Wp.tile([128, 4, 16], dt.float8e4, name="wpc8t",
                            tag="wpc8t")
            nc.scalar.dma_start(out=wpc8t, in_=wpc8_d[:])
            idb = Wp.tile([128, 128], dt.bfloat16, name="idb", tag="idb")
            nc.scalar.dma_start(out=idb, in_=idb_d[:])
            selt = Wp.tile([8, 4, 128], dt.bfloat16, name="selt", tag="selt")
            nc.scalar.dma_start(out=selt, in_=sel_d[:])
            for i in (0,):
                nc.scalar.dma_start(out=pmTt[:, i, :], in_=pmT_d[:, i, :])
                nc.scalar.dma_start(out=pmt[:, i, :], in_=pm_d[:, i, :])
                nc.scalar.dma_start(out=maddt[:, i, :], in_=madd_d[:, i, :])
            # gpsimd: wv8, xq, masks for qt=2/3
            wv8t = Wp.tile([128, 4, E], dt.float8e4, name="wv8t", tag="wv8t")
            nc.gpsimd.dma_start(out=wv8t, in_=wv8_d[:])
            xq_all = Wp.tile([128, 4, E], dt.bfloat16, name="xq_all",
                             tag="xq_all")
            nc.gpsimd.dma_start(out=xq_all, in_=xq_d[:])
            for i in range(2, 4):
                nc.gpsimd.dma_start(out=pmTt[:, i, :], in_=pmT_d[:, i, :])
                nc.gpsimd.dma_start(out=pmt[:, i, :], in_=pm_d[:, i, :])
                nc.gpsimd.dma_start(out=maddt[:, i, :], in_=madd_d[:, i, :])
            pmT = [pmTt[:, i, :] for i in range(4)]
            pm = [pmt[:, i, :] for i in range(4)]
            madd = [maddt[:, i, :] for i in range(4)]
            xq = [xq_all[:, qt, :] for qt in range(4)]

            # small bias tiles (scalar queue), only when actually used
            def sload(dram, shape, dtype, name, n=None):
                if n is None:
                    t = Wp.tile(shape, dtype, name=name, tag=name)
                    nc.scalar.dma_start(out=t, in_=dram[:])
                    return t
                t = Wp.tile([128, n, shape[1]], dtype, name=name, tag=name)
                nc.scalar.dma_start(out=t, in_=dram[:])
                return [t[:, i, :] for i in range(n)]

            need_ones = not (zpc and zv and zo and z2f)
            ones1 = sload(ones_d, [1, 128], dt.bfloat16, "ones1") if need_ones else None
            bpc = sload(bpc_d, [1, 16], dt.bfloat16, "bpc") if not zpc else None
            bqc = sload(bqc_d, [128, 1], dt.float32, "bqc", 4) if not zq else None
            bkc = sload(bkc_d, [128, 1], dt.float32, "bkc", 4) if not zk else None
            bvr = sload(bvr_d, [1, E], dt.bfloat16, "bvr") if not zv else None
            bor = sload(bor_d, [1, E], dt.bfloat16, "bor") if not zo else None
            b2r = sload(b2r_d, [1, E], dt.bfloat16, "b2r") if not z2f else None
            b1c = sload(b1c_d, [128, 1], dt.float32, "b1c", 16) if not zb1 else None
            b1c16 = sload(b1c16_d, [128, 1], dt.float32, "b1c16", 16) if not zb1 else None
            g1c = sload(g1c_d, [128, 1], dt.float32, "g1c", 4) if not ln1t else None
            b1lc = sload(b1lc_d, [128, 1], dt.float32, "b1lc", 4) if not ln1t else None
            eps = Wp.tile([128, 1], dt.float32, name="eps", tag="eps")
            nc.vector.memset(eps, 1e-5)

            MM = nc.tensor.matmul

            def MM8(out, lhsT, rhs, start, stop):
                MM(out, lhsT, rhs, start=start, stop=stop, perf_mode=DR)

            nalt = [0]
            IVS = 1.0 / WS

            def ps2sb(out, ps, scale=None, bias=None):
                """psum->sbuf eviction, alternating DVE/ACT; optional
                (ps*scale)+bias with per-partition bias."""
                nalt[0] += 1
                if scale is not None and bias is not None:
                    nc.vector.tensor_scalar(out, ps, scale, bias,
                                            OP.mult, OP.add)
                elif scale is not None:
                    if nalt[0] % 2 == 0:
                        nc.vector.tensor_scalar(out, ps, scale, None, OP.mult)
                    else:
                        nc.scalar.activation(out, ps, AF.Copy, scale=scale)
                elif nalt[0] % 2 == 0:
                    nc.vector.tensor_copy(out, ps)
                else:
                    nc.scalar.copy(out, ps)

            # ---- Phase A: pcb + q/k/v projections (fp8 DoubleRow) ----
            qT, kT, pcb = [], [], []
            v = [None] * 8
            with tc.tile_pool(name="psA", bufs=2, space="PSUM") as psA:
                for qt in range(4):
                    ps = psA.tile([128, 16], dt.float32, name=f"pspcb{qt}",
                                  tag="pspcb")
                    for c in range(2):
                        MM8(ps, xT8q[c][:, :, qt * 128:(qt + 1) * 128],
                            wpc8t[:, 2 * c:2 * c + 2, :],
                            start=(c == 0), stop=(zpc and c == 1))
                    if not zpc:
                        MM(ps, ones1, bpc, start=False, stop=True)
                    t = Wp.tile([128, 16], dt.float32, name=f"pcb{qt}",
                                tag=f"pcb{qt}")
                    nc.vector.tensor_scalar(t, ps, IVS, None, OP.mult)
                    pcb.append(t)
                for m in range(4):
                    ps = psA.tile([128, NQ], dt.float32, name=f"psq{m}",
                                  tag="psq")
                    for c in range(2):
                        MM8(ps, wq8t[:, 2 * c:2 * c + 2,
                                     m * 128:(m + 1) * 128],
                            xT8q[c], start=(c == 0), stop=(c == 1))
                    t = Wp.tile([128, NQ], dt.bfloat16, name=f"qT{m}",
                                tag=f"qT{m}")
                    # fold 1/sqrt(HD)=1/8 and the 1/16 weight scale here
                    if zq:
                        nc.vector.tensor_scalar(t, ps, IVS / 8.0, None,
                                                OP.mult)
                    else:
                        nc.vector.tensor_scalar(t, ps, IVS / 8.0, bqc[m],
                                                OP.mult, OP.add)
                    qT.append(t)
                    tk = Wp.tile([128, N], dt.bfloat16, name=f"kT{m}",
                                 tag=f"kT{m}")
                    psk = psA.tile([128, N], dt.float32, name=f"psk{m}",
                                   tag="psk")
                    for c in range(2):
                        for tb in range(2):
                            sl = slice(tb * 512, tb * 512 + 512)
                            MM8(psk[:, sl],
                                wk8t[:, 2 * c:2 * c + 2,
                                     m * 128:(m + 1) * 128],
                                xT8t[:, 2 * c:2 * c + 2, sl],
                                start=(c == 0), stop=(c == 1))
                    for tb in range(2):
                        sl = slice(tb * 512, tb * 512 + 512)
                        ps2sb(tk[:, sl], psk[:, sl], scale=IVS,
                              bias=None if zk else bkc[m])
                    kT.append(tk)
                # v-projection
                for tt in range(8):
                    psv = psA.tile([128, E], dt.float32, name=f"psv{tt}",
                                   tag="psq")
                    for c in range(2):
                        MM8(psv, xT8t[:, 2 * c:2 * c + 2,
                                      tt * 128:(tt + 1) * 128],
                            wv8t[:, 2 * c:2 * c + 2, :],
                            start=(c == 0), stop=(zv and c == 1))
                    if not zv:
                        MM(psv, ones1, bvr, start=False, stop=True)
                    t = Wp.tile([128, E], dt.bfloat16, name=f"v{tt}",
                                tag=f"v{tt}")
                    ps2sb(t, psv, scale=IVS)
                    v[tt] = t
            # late weight loads: gated on v so the 2MB doesn't steal
            # DMA bandwidth from the startup-critical loads
            gate = sm.tile([128, 1], dt.bfloat16, name="gate", tag="gate")
            nc.gpsimd.tensor_copy(gate, v[7][:, 0:1])
            wo8t = Wp.tile([128, 4, E], dt.float8e4, name="wo8t", tag="wo8t")
            nc.gpsimd.dma_start(out=wo8t, in_=wo8_d[:])
            w18t = Wp.tile([128, 4, F], dt.float8e4, name="w18t", tag="w18t")
            nc.gpsimd.dma_start(out=w18t, in_=w18_d[:])
            w28ht = Wp.tile([128, 8, E], dt.float8e4, name="w28ht",
                            tag="w28ht")
            nc.gpsimd.dma_start(out=w28ht, in_=w28h_d[:])
            w2bt = Wp.tile([128, 8, E], dt.bfloat16, name="w2bt", tag="w2bt")
            nc.gpsimd.dma_start(out=w2bt, in_=w2b_d[:])
            w2b = [w2bt[:, i, :] for i in range(8)]

            # ---- Phase B: attention (bf16 scores + PV) ----
            ctxT8 = Wp.tile([128, 4, NQ], dt.float8e4, name="ctxT8",
                            tag="ctxT8")
            with (tc.tile_pool(name="psS", bufs=3, space="PSUM") as psS,
                  tc.tile_pool(name="psX", bufs=2, space="PSUM") as psX):
                pT_pend = [None] * 4

                ts_pend = [None] * 4

                def emit_ctx(m):
                    ps_ctx = psX.tile([128, NQ], dt.float32, name=f"psctx{m}",
                                      tag="psctx", bufs=1)
                    pT_all = pT_pend[m]
                    for hh in range(2):
                        h = 2 * m + hh
                        po = hh * 64
                        for kb in range(8):
                            MM(ps_ctx[po:po + 64, :],
                               v[kb][:, h * 64:(h + 1) * 64],
                               pT_all[:, hh * 8 + kb, :], start=(kb == 0),
                               stop=(kb == 7))
                    # normalize whole 2-head block at eviction: ctx * (1/s)
                    nc.vector.tensor_tensor(ctxT8[:, m, :], ps_ctx,
                                            ts_pend[m], OP.mult)

                for m in range(4):
                    # pT_all[:, hh*8+kb, q] = P_raw[h=2m+hh][q, kb*128+p]
                    pT_all = sc.tile([128, 16, NQ], dt.bfloat16,
                                     name=f"pTall{m}", tag="pTall", bufs=2)
                    pT_pend[m] = pT_all
                    smat = sc.tile([128, 128], dt.bfloat16, name=f"smat{m}",
                                   tag="smat", bufs=2)
                    nc.vector.memset(smat, 0.0)
                    for qt in range(4):
                        pn = sc.tile([128, 2, N], dt.bfloat16,
                                     name=f"pn_{m}_{qt}", tag="pn", bufs=2)
                        for hh in range(2):
                            h = 2 * m + hh
                            pb = pcb[qt][:, h:h + 1]
                            cb = pcb[qt][:, 8 + h:9 + h]
                            # am = pm*cb + pmT*pb via fast 2x-mode DVE ops;
                            # madd is injected on the tensor engine
                            t1 = sc.tile([128, N], dt.bfloat16,
                                         name=f"t1_{h}_{qt}", tag=f"t1_{hh}",
                                         bufs=1)
                            nc.vector.tensor_scalar(t1, pm[qt], cb, None,
                                                    OP.mult)
                            t2 = sc.tile([128, N], dt.bfloat16,
                                         name=f"t2_{h}_{qt}", tag=f"t2_{hh}",
                                         bufs=1)
                            nc.vector.tensor_scalar(t2, pmT[qt], pb, None,
                                                    OP.mult)
                            am = sc.tile([128, N], dt.bfloat16,
                                         name=f"am_{h}_{qt}", tag=f"am_{hh}",
                                         bufs=2)
                            nc.vector.tensor_tensor(am, t1, t2, OP.add)
                            ps_s = psS.tile([128, N], dt.float32,
                                            name=f"pss_{h}_{qt}", tag="ps_s")
                            for tb in range(2):
                                sl = slice(tb * 512, tb * 512 + 512)
                                MM(ps_s[:, sl], idb, am[:, sl],
                                   start=True, stop=False)
                                MM(ps_s[:, sl], idb, madd[qt][:, sl],
                                   start=False, stop=False)
                                MM(ps_s[:, sl],
                                   qT[m][hh * 64:hh * 64 + 64,
                                         qt * 128:(qt + 1) * 128],
                                   kT[m][hh * 64:hh * 64 + 64, sl],
                                   start=False, stop=(tb == 1))
                            sums = sm.tile([128, 1], dt.float32,
                                           name=f"sums_{h}_{qt}", tag="sums")
                            nc.scalar.activation(pn[:, hh, :], ps_s, AF.Exp,
                                                 accum_out=sums)
                            with nc.allow_low_precision(
                                    reason="1/s in bf16; uniform per-row "
                                           "scale, tolerance 2e-2"):
                                nc.vector.reciprocal(
                                    smat[:, hh * 4 + qt:hh * 4 + qt + 1],
                                    sums)
                            nc.sync.dma_start_transpose(
                                out=pT_all[:, hh * 8:hh * 8 + 8,
                                           qt * 128:(qt + 1) * 128],
                                in_=pn[:, hh, :])
                    if m > 0:
                        emit_ctx(m - 1)
                    # t_s[p, q] = 1/s_{head(p)}[q], broadcast via matmul
                    smatT = sc.tile([128, 128], dt.bfloat16, name=f"smatT{m}",
                                    tag="smatT", bufs=2)
                    nc.sync.dma_start_transpose(out=smatT, in_=smat)
                    ps_ts = psX.tile([128, NQ], dt.float32, name=f"psts{m}",
                                     tag="psts", bufs=1)
                    for qt in range(4):
                        MM(ps_ts[:, qt * 128:(qt + 1) * 128],
                           selt[:, qt, :], smatT[0:8, :],
                           start=True, stop=True)
                    t_s = sc.tile([128, NQ], dt.float32, name=f"ts{m}",
                                  tag="ts", bufs=2)
                    nc.scalar.copy(t_s, ps_ts)
                    ts_pend[m] = t_s
                emit_ctx(3)

            # ---- Phase C1: Wo + LN1 + y transpose ----
            yb = []
            yT8 = Wp.tile([128, 4, NQ], dt.float8e4, name="yT8", tag="yT8")
            with (tc.tile_pool(name="psAO", bufs=2, space="PSUM") as psAO,
                  tc.tile_pool(name="psYT", bufs=1, space="PSUM") as psYT):
                ps_yT = psYT.tile([128, 4 * NQ], dt.float32, name="ps_yT",
                                  tag="ps_yT")
                for qt in range(4):
                    ps_ao = psAO.tile([128, E], dt.float32, name=f"psao{qt}",
                                      tag="ps_ao")
                    for c in range(2):
                        MM8(ps_ao, ctxT8[:, 2 * c:2 * c + 2,
                                         qt * 128:(qt + 1) * 128],
                            wo8t[:, 2 * c:2 * c + 2, :],
                            start=(c == 0), stop=(zo and c == 1))
                    if not zo:
                        MM(ps_ao, ones1, bor, start=False, stop=True)
                    z = ln.tile([128, E], dt.float32, name=f"z{qt}", tag="z")
                    nc.vector.scalar_tensor_tensor(z, ps_ao, IVS, xq[qt],
                                                   OP.mult, OP.add)
                    stats = sm.tile([128, nc.vector.BN_STATS_DIM], dt.float32,
                                    name=f"stats{qt}", tag="stats")
                    nc.vector.bn_stats(out=stats, in_=z)
                    mv = sm.tile([128, nc.vector.BN_AGGR_DIM], dt.float32,
                                 name=f"mv{qt}", tag="mv")
                    nc.vector.bn_aggr(out=mv, in_=stats)
                    sd = sm.tile([128, 1], dt.float32, name=f"sd{qt}",
                                 tag="sd")
                    nc.scalar.activation(sd, mv[:, 1:2], AF.Sqrt, bias=eps)
                    rstd = sm.tile([128, 1], dt.float32, name=f"rstd{qt}",
                                   tag="rstd")
                    nc.vector.reciprocal(rstd, sd)
                    t = Wp.tile([128, E], dt.bfloat16, name=f"yb{qt}",
                                tag=f"yb{qt}")
                    nc.vector.tensor_scalar(t, z, mv[:, 0:1], rstd,
                                            OP.subtract, OP.mult)
                    yb.append(t)
                    for ec in range(4):
                        MM(ps_yT[:, ec * NQ + qt * 128:
                                 ec * NQ + (qt + 1) * 128],
                           t[:, ec * 128:(ec + 1) * 128], idb,
                           start=True, stop=True)
                for ec in range(4):
                    if ln1t:
                        ps2sb(yT8[:, ec, :], ps_yT[:, ec * NQ:(ec + 1) * NQ])
                    else:
                        nc.vector.tensor_scalar(
                            yT8[:, ec, :], ps_yT[:, ec * NQ:(ec + 1) * NQ],
                            g1c[ec], b1lc[ec], OP.mult, OP.add)

            # ---- Phase C2: FFN + LN2 ----
            h18 = Wp.tile([128, 8, NQ], dt.float8e4, name="h18", tag="h18")
            h1b = []
            with (tc.tile_pool(name="psH", bufs=3, space="PSUM") as psH,
                  tc.tile_pool(name="psF", bufs=2, space="PSUM") as psF):
                for fo in range(16):
                    ps = psH.tile([128, NQ], dt.float32, name=f"psh{fo}",
                                  tag="psH")
                    for c in range(2):
                        MM8(ps, w18t[:, 2 * c:2 * c + 2,
                                     fo * 128:(fo + 1) * 128],
                            yT8[:, 2 * c:2 * c + 2, :],
                            start=(c == 0), stop=(c == 1))
                    if fo < 8:
                        # fp8 h1 at 1x (psum holds 16x)
                        if zb1:
                            if fo % 2 == 0:
                                nc.vector.tensor_scalar(h18[:, fo, :], ps,
                                                        0.0, IVS,
                                                        OP.max, OP.mult)
                            else:
                                nc.scalar.activation(h18[:, fo, :], ps,
                                                     AF.Relu, scale=IVS)
                        else:
                            nc.scalar.activation(h18[:, fo, :], ps, AF.Relu,
                                                 scale=IVS, bias=b1c[fo])
                    else:
                        # bf16 h1 kept at 16x so FFN2 psum scales match
                        th = Wp.tile([128, NQ], dt.bfloat16,
                                     name=f"h1b_{fo}", tag=f"h1b_{fo}")
                        if zb1:
                            if fo % 2 == 0:
                                nc.vector.tensor_scalar(th, ps, 0.0, None,
                                                        OP.max)
                            else:
                                nc.scalar.activation(th, ps, AF.Relu)
                        else:
                            nc.scalar.activation(th, ps, AF.Relu,
                                                 bias=b1c16[fo])
                        h1b.append(th)
                for qt in range(4):
                    ps_ff = psF.tile([128, E], dt.float32, name=f"psff{qt}",
                                     tag="psF")
                    for fc in range(4):
                        MM8(ps_ff, h18[:, 2 * fc:2 * fc + 2,
                                       qt * 128:(qt + 1) * 128],
                            w28ht[:, 2 * fc:2 * fc + 2, :],
                            start=(fc == 0), stop=False)
                    for fc in range(8):
                        MM(ps_ff, h1b[fc][:, qt * 128:(qt + 1) * 128],
                           w2b[fc], start=False, stop=(z2f and fc == 7))
                    if not z2f:
                        MM(ps_ff, ones1, b2r, start=False, stop=True)
                    # residual add folded into the eviction
                    z2 = ln.tile([128, E], dt.float32, name=f"z2_{qt}",
                                 tag="z2")
                    nc.vector.scalar_tensor_tensor(z2, ps_ff, IVS, yb[qt],
                                                   OP.mult, OP.add)
                    stats2 = sm.tile([128, nc.vector.BN_STATS_DIM],
                                     dt.float32, name=f"stats2_{qt}",
                                     tag="stats2")
                    nc.vector.bn_stats(out=stats2, in_=z2)
                    mv2 = sm.tile([128, nc.vector.BN_AGGR_DIM], dt.float32,
                                  name=f"mv2_{qt}", tag="mv2")
                    nc.vector.bn_aggr(out=mv2, in_=stats2)
                    sd2 = sm.tile([128, 1], dt.float32, name=f"sd2_{qt}",
                                  tag="sd2")
                    nc.scalar.activation(sd2, mv2[:, 1:2], AF.Sqrt, bias=eps)
                    rstd2 = sm.tile([128, 1], dt.float32, name=f"rstd2_{qt}",
                                    tag="rstd2")
                    nc.vector.reciprocal(rstd2, sd2)
                    outf = ln.tile([128, E], dt.float32, name=f"outf{qt}",
                                   tag="outf")
                    nc.vector.tensor_scalar(outf, z2, mv2[:, 0:1], rstd2,
                                            OP.subtract, OP.mult)
                    nc.sync.dma_start(out=out_d[qt], in_=outf)

    nc.compile()
    return nc


def _shard(inputs):
    f32 = np.float32
    x = np.asarray(inputs["node_inputs"], f32)
    pmk = np.asarray(inputs["parent_mask"], f32)
    hid = np.asarray(inputs["hidden"]).astype(bool)
    pad = np.asarray(inputs["pad_mask"]).astype(bool)
    Wqkv = np.asarray(inputs["Wqkv"], f32)
    bqkv = np.asarray(inputs["bqkv"], f32)
    Wq, Wk, Wv = Wqkv[:E], Wqkv[E:2 * E], Wqkv[2 * E:]
    bq, bk, bv = bqkv[:E], bqkv[E:2 * E], bqkv[2 * E:]

    def tobf(a):
        return np.ascontiguousarray(a, dtype=f32).astype(BF)

    def to8(a, chunks, width):
        """[E_in, width] -> fp8 x16, chunked [128, chunks, width]."""
        return np.ascontiguousarray(
            (np.ascontiguousarray(a, dtype=f32) * WS).astype(F8)
            .reshape(chunks, 128, width).transpose(1, 0, 2))

    shared = {
        "wq8": to8(Wq.T, 4, E),
        "wk8": to8(Wk.T, 4, E),
        "wv8": to8(Wv.T, 4, E),
        "wpc8": to8(np.concatenate([np.asarray(inputs["Wp"], f32),
                                    np.asarray(inputs["Wc"], f32)], 0).T,
                    4, 16),
        "wo8": to8(np.asarray(inputs["Wo"], f32).T, 4, E),
        "w18": to8(np.asarray(inputs["W1"], f32).T, 4, F),
        "w28h": to8(np.asarray(inputs["W2"], f32)[:, :F // 2].T, 8, E),
        "w2b": np.ascontiguousarray(
            tobf(np.asarray(inputs["W2"], f32)[:, F // 2:].T)
            .reshape(8, 128, E).transpose(1, 0, 2)),
        "bpc": tobf(np.concatenate([np.asarray(inputs["bp"], f32),
                                    np.asarray(inputs["bc"], f32)])[None]
                    * WS),
        "bor": tobf(np.asarray(inputs["bo"], f32)[None] * WS),
        "b2r": tobf(np.asarray(inputs["b2"], f32)[None] * WS),
        "bvr": tobf(bv[None] * WS),
        "b1c": np.ascontiguousarray(
            np.asarray(inputs["b1"], f32).reshape(16, 128, 1).transpose(1, 0, 2)),
        "b1c16": np.ascontiguousarray(
            (np.asarray(inputs["b1"], f32) * WS).reshape(16, 128, 1)
            .transpose(1, 0, 2)),
        "bqc": np.ascontiguousarray((bq / 8.0).reshape(4, 128, 1).transpose(1, 0, 2)),
        "bkc": np.ascontiguousarray(bk.reshape(4, 128, 1).transpose(1, 0, 2)),
        "g1c": np.ascontiguousarray(
            np.asarray(inputs["ln1_g"], f32).reshape(4, 128, 1).transpose(1, 0, 2)),
        "b1lc": np.ascontiguousarray(
            np.asarray(inputs["ln1_b"], f32).reshape(4, 128, 1).transpose(1, 0, 2)),
        "idb": np.eye(128, dtype=BF),
        "ones1": np.ones((1, 128), BF),
        "sel": np.ascontiguousarray(
            (np.arange(8)[:, None, None]
             == (np.arange(128)[None, None, :] // 64) * 4
             + np.arange(4)[None, :, None]).astype(BF)),
    }
    in_maps = []
    for c in range(NCORES):
        b_i, qh = c // 2, c % 2
        qo = qh * NQ
        # key-token permutation: own q-half first (so xT8q == xT8[:, :NQ])
        perm = np.concatenate([np.arange(qo, qo + NQ),
                               np.arange(0, qo), np.arange(qo + NQ, N)])
        xb = x[:, b_i, :]
        xTp = np.ascontiguousarray(xb.T[:, perm], dtype=f32)
        m = dict(shared)
        m["xT8"] = np.ascontiguousarray(
            xTp.astype(F8).reshape(4, 128, N).transpose(1, 0, 2))
        m["xq"] = np.ascontiguousarray(
            xb[qo:qo + NQ].astype(BF).reshape(4, 128, E).transpose(1, 0, 2))
        m["pm"] = np.ascontiguousarray(
            tobf(pmk[b_i][np.ix_(np.arange(qo, qo + NQ), perm)]
                 ).reshape(4, 128, N).transpose(1, 0, 2))
        m["pmT"] = np.ascontiguousarray(
            tobf(pmk[b_i][np.ix_(perm, np.arange(qo, qo + NQ))].T
                 ).reshape(4, 128, N).transpose(1, 0, 2))
        m["madd"] = np.ascontiguousarray(np.where(
            hid[b_i][np.ix_(np.arange(qo, qo + NQ), perm)]
            | pad[b_i][perm][None, :],
            f32(-1e30), f32(0)).astype(BF).reshape(4, 128, N).transpose(1, 0, 2))
        in_maps.append(m)
    return in_maps


def kernel(**inputs):
    from concourse.bass_utils import run_bass_kernel_spmd

    def _z(name):
        return bool(np.all(np.asarray(inputs[name]) == 0))

    flags = dict(
        zq=_z("bqkv"), zk=_z("bqkv"), zv=_z("bqkv"),
        zpc=_z("bp") and _z("bc"), zo=_z("bo"), z2f=_z("b2"),
        zb1=_z("b1"),
        ln1t=bool(np.all(np.asarray(inputs["ln1_g"]) == 1.0)
                  and np.all(np.asarray(inputs["ln1_b"]) == 0.0)))
    key = ("nc",) + tuple(sorted(flags.items()))
    nc = _CACHE.get(key)
    if nc is None:
        nc = _build_nc(**flags)
        _CACHE[key] = nc
    in_maps = _shard(inputs)
    trace = _CACHE.get("trace", False)
    res = run_bass_kernel_spmd(nc, in_maps, core_ids=list(range(NCORES)),
                               trace=trace,
                               tmpdir=_CACHE.get("tmpdir"))
    _CACHE["last_result"] = res

    out = np.zeros((N, B, E), np.float32)
    for c in range(NCORES):
        b_i, qh = c // 2, c % 2
        qo = qh * NQ
        out[qo:qo + NQ, b_i, :] = res.results[c]["out"].reshape(NQ, E)

    g2 = np.asarray(inputs["ln2_g"], np.float32)
    b2l = np.asarray(inputs["ln2_b"], np.float32)
    if not (np.all(g2 == 1.0) and np.all(b2l == 0.0)):
        out = out * g2 + b2l
    return out
